# revision 1
# baseline (speedup 1.0000x reference)
"""Trainium2 Bass kernel for nn_BackupBarrierCBF.

Reference semantics (B=1024, A=64, T=50 unicycle rollout + rect-vs-disc
distance + min-over-horizon + saturation). Crucial subtleties:
  - braking controller: u = (-9*tanh(2*v), 0) => theta is CONSTANT, so
    positions are x0 + cos(theta)*dt*cumsum(v).
  - veh_veh_distance receives traj[..., 0:3] = (x, y, v): the body-frame
    rotation angle is the (time-varying) VELOCITY, not theta.
  - traj slot k holds the state AFTER k+1 steps: position cumsum uses
    v_0..v_k while the stored rotation angle is v_{k+1}.

Per-core structure (batch rows on the 128 partitions):
  - 50-step serial v-recurrence (ACT Tanh + DVE scalar_tensor_tensor)
    writing straight into a t-major trajectory (all chain ops contiguous);
    the col-major cumsum ST is built by per-step adds and the angle range
    reduction runs in the rollout's DVE slack. Constants precede the
    rollout so their ACT Sins/Sqrts don't thrash the Tanh table.
  - sin/cos of v(t) on ACT with col-major STRIDED writes (2.2x ACT penalty,
    but ACT has slack and every later DVE op stays unit-stride). Range
    reduction only for the first k_red slots (|v| provably <= pi afterward:
    while |v|>2.2 each step shrinks |v| by >= 0.8997 and the map keeps
    |v| <= pi once below). cos x = sin(pi/2 - |x|).
  - distance phase: ~28 big [128, 64, 50] DVE ops, a-major, unit inner
    stride, per-agent constants broadcast with 0-step APs; SINV-products
    ordered first (COSV finishes later on ACT); abs on ACT, fine-grained.
  - NO gpsimd tensor work: gpsimd shares the DVE SBUF port (measured 2.5x
    DVE slowdown when overlapped - net zero).

Sharding: pure data parallel over batch B across 8 cores (128 rows/core).
"""
import numpy as np
import concourse.bass as bass
import concourse.bacc as bacc
import concourse.tile as tile
from concourse import mybir
from concourse.bass_utils import run_bass_kernel_spmd

F32 = mybir.dt.float32
I32 = mybir.dt.int32
OP = mybir.AluOpType
ACT = mybir.ActivationFunctionType

B, A, F = 1024, 64, 15
N_CORES = 8
PB = B // N_CORES          # 128 batch rows per core (partition dim)
T = 50
NC2 = 2 * A                # 128 columns: [ego agents | other agents]
NT = T * A                 # 3200
TWO_PI = float(2.0 * np.pi)

_cache: dict = {}


def _ap(t: bass.AP, extra_offset: int, free_dims: list) -> bass.AP:
    """View into tile t: keep partition dim, replace free dims."""
    return bass.AP(tensor=t.tensor, offset=t.offset + extra_offset,
                   ap=[list(t.ap[0])] + [list(d) for d in free_dims])


def _build(dt_uniform, k_red):
    nc = bacc.Bacc("TRN2", target_bir_lowering=False)
    data = nc.dram_tensor("data", [PB, A * F], F32, kind="ExternalInput")
    out = nc.dram_tensor("out", [PB, A], F32, kind="ExternalOutput")

    with tile.TileContext(nc) as tc:
        with tc.tile_pool(name="pool", bufs=1) as pool:
            # ---------------- load ----------------
            D = pool.tile([PB, A * F], F32)
            nc.sync.dma_start(out=D[:], in_=data[:])

            def fld(k):  # [128, 64] strided view of per-agent field k
                return _ap(D, k, [[F, A]])

            halfpi = pool.tile([PB, 1], F32)
            nc.vector.memset(halfpi[:], float(np.pi / 2))

            cons = pool.tile([PB, 12, A], F32)

            def c(i):
                return _ap(cons, i * A, [[1, A]])

            def cb(i):  # broadcast over inner t: [128, 64, T]
                return _ap(cons, i * A, [[1, A], [0, T]])

            C_P0X, C_P0Y = 0, 1
            C_D1, C_D2, C_D3, C_K2Y = 2, 3, 4, 5
            C_CEDT, C_SEDT, C_CADT, C_SADT = 6, 7, 8, 9
            C_RE, C_RA = 10, 11

            scr = pool.tile([PB, 10, A], F32)

            def s(i):
                return _ap(scr, i * A, [[1, A]])

            ki = pool.tile([PB, 4, A], I32)

            def kis(i):
                return _ap(ki, i * A, [[1, A]])

            # ---------------- per-agent constants (front) ------------
            # Their ACT Sins/Sqrts run before any Tanh so the ACT table is
            # loaded once per function; four separate scratches keep the
            # sincos pipelines independent.
            def sincos(theta_ap, out_sin, out_cos, base):
                for idx, (want_cos, dst) in enumerate(((False, out_sin),
                                                       (True, out_cos))):
                    sc = s(base + idx)
                    shift = 0.25 if want_cos else 0.0
                    nc.vector.tensor_scalar(out=sc, in0=theta_ap,
                                            scalar1=1.0 / TWO_PI, scalar2=shift,
                                            op0=OP.mult, op1=OP.add)
                    nc.vector.tensor_copy(out=kis(base + idx), in_=sc)
                    nc.vector.tensor_copy(out=sc, in_=kis(base + idx))
                    nc.vector.scalar_tensor_tensor(
                        out=sc, in0=sc, scalar=-TWO_PI, in1=theta_ap,
                        op0=OP.mult, op1=OP.add)
                    nc.scalar.activation(
                        out=dst, in_=sc, func=ACT.Sin,
                        bias=halfpi[:] if want_cos else 0.0, scale=1.0)

            sincos(fld(7), c(C_SADT), c(C_CADT), 0)
            sincos(fld(3), c(C_SEDT), c(C_CEDT), 2)
            for i in (C_CADT, C_SADT, C_CEDT, C_SEDT):
                nc.vector.tensor_mul(out=c(i), in0=c(i), in1=fld(14))

            nc.vector.tensor_mul(out=s(4), in0=fld(8), in1=fld(8))
            nc.vector.tensor_mul(out=s(5), in0=fld(9), in1=fld(9))
            nc.vector.tensor_add(out=s(4), in0=s(4), in1=s(5))
            nc.scalar.activation(out=c(C_RE), in_=s(4), func=ACT.Sqrt,
                                 scale=0.25)
            nc.vector.tensor_mul(out=s(6), in0=fld(11), in1=fld(11))
            nc.vector.tensor_mul(out=s(7), in0=fld(12), in1=fld(12))
            nc.vector.tensor_add(out=s(6), in0=s(6), in1=s(7))
            nc.scalar.activation(out=c(C_RA), in_=s(6), func=ACT.Sqrt,
                                 scale=0.25)
            # d1 = 0.5*(We-Le); d2 = 0.5*(Wa-La); k1y = 0.5*We+ra;
            # k2y = 0.5*Wa+re; d3 = k2y-k1y
            nc.vector.tensor_sub(out=s(8), in0=fld(9), in1=fld(8))
            nc.vector.tensor_scalar_mul(out=c(C_D1), in0=s(8), scalar1=0.5)
            nc.vector.tensor_sub(out=s(9), in0=fld(12), in1=fld(11))
            nc.vector.tensor_scalar_mul(out=c(C_D2), in0=s(9), scalar1=0.5)
            nc.vector.scalar_tensor_tensor(
                out=s(8), in0=fld(9), scalar=0.5, in1=c(C_RA),
                op0=OP.mult, op1=OP.add)          # k1y
            nc.vector.scalar_tensor_tensor(
                out=c(C_K2Y), in0=fld(12), scalar=0.5, in1=c(C_RE),
                op0=OP.mult, op1=OP.add)          # k2y
            nc.vector.tensor_sub(out=c(C_D3), in0=c(C_K2Y), in1=s(8))
            nc.vector.tensor_sub(out=c(C_P0X), in0=fld(4), in1=fld(0))
            nc.vector.tensor_sub(out=c(C_P0Y), in0=fld(5), in1=fld(1))

            # ---------------- rollout ----------------
            # Serial chain writes straight into t-major VT (slot j at
            # j*NC2); col-major ST built by per-step adds; the angle
            # range-reduce fills the rollout's DVE slack.
            VT = pool.tile([PB, (T + 1) * NC2], F32, tag="tVT")
            ST = pool.tile([PB, NC2 * T], F32, tag="tST")

            def vslot(j):  # j=0: strided input view; j in 1..50: contiguous
                if j == 0:
                    return _ap(D, 2, [[4, 2], [F, A]])
                return _ap(VT, j * NC2, [[1, NC2]])

            def stslot(k):  # k in 0..49, col-major strided
                return _ap(ST, k, [[T, NC2]])

            G = pool.tile([PB, NC2], F32)
            nc.vector.tensor_copy(out=stslot(0), in_=vslot(0))

            if dt_uniform is None:
                NDT2 = pool.tile([PB, NC2], F32)
                nc.vector.tensor_scalar_mul(
                    out=NDT2[:], in0=_ap(D, 14, [[0, 2], [F, A]]), scalar1=-9.0)

            MS = KI2 = None
            if k_red > 0:
                MS = pool.tile([PB, NC2 * k_red], F32, tag="tPXY")
                KI2 = pool.tile([PB, NC2 * k_red], I32, tag="tSCR")

            SEv = _ap(ST, 0, [[T, A], [1, T]])
            SAv = _ap(ST, A * T, [[T, A], [1, T]])
            PXY = pool.tile([PB, 2 * NT], F32, tag="tPXY")
            SCR = pool.tile([PB, 2 * NT], F32, tag="tSCR")

            for j in range(1, T + 1):
                nc.scalar.activation(out=G[:], in_=vslot(j - 1),
                                     func=ACT.Tanh, scale=2.0)
                if dt_uniform is None:
                    nc.vector.tensor_mul(out=G[:], in0=G[:], in1=NDT2[:])
                    nc.vector.tensor_add(out=vslot(j), in0=vslot(j - 1),
                                         in1=G[:])
                else:
                    nc.vector.scalar_tensor_tensor(
                        out=vslot(j), in0=G[:], scalar=-9.0 * float(dt_uniform),
                        in1=vslot(j - 1), op0=OP.mult, op1=OP.add)
                if j < T:
                    nc.vector.tensor_add(out=stslot(j), in0=stslot(j - 1),
                                         in1=vslot(j))
                if j == k_red and k_red > 0:
                    # range-reduce angle slots 1..k_red in place (all
                    # ST-adds reading the raw values already emitted)
                    red_view = _ap(VT, NC2, [[1, NC2 * k_red]])
                    nc.vector.tensor_scalar_mul(out=MS[:], in0=red_view,
                                                scalar1=1.0 / TWO_PI)
                    nc.vector.tensor_copy(out=KI2[:], in_=MS[:])
                    nc.vector.tensor_copy(out=MS[:], in_=KI2[:])
                    nc.vector.scalar_tensor_tensor(
                        out=red_view, in0=MS[:], scalar=-TWO_PI, in1=red_view,
                        op0=OP.mult, op1=OP.add)

            # ---------------- trig of v (angles are v_{k+1}) ----------
            # t-major contiguous reads, col-major strided writes (ACT has
            # slack; DVE consumers stay unit-stride).  SINV first so the
            # rel phase's SINV-products can start earliest.
            ang = _ap(VT, NC2, [[1, T * NC2]])
            SINV = pool.tile([PB, NC2 * T], F32)
            COSV = pool.tile([PB, NC2 * T], F32)
            cm_out_sin = _ap(SINV, 0, [[1, T], [T, NC2]])
            cm_out_cos = _ap(COSV, 0, [[1, T], [T, NC2]])
            nc.scalar.activation(out=cm_out_sin, in_=ang, func=ACT.Sin)
            nc.scalar.activation(out=cm_out_cos, in_=ang, func=ACT.Abs)
            nc.scalar.activation(out=COSV[:], in_=COSV[:], func=ACT.Sin,
                                 bias=halfpi[:], scale=-1.0)

            S1 = _ap(SCR, 0, [[1, NT]])
            S2 = _ap(SCR, NT, [[1, NT]])
            PX = _ap(PXY, 0, [[1, NT]])
            PY = _ap(PXY, NT, [[1, NT]])

            nc.vector.tensor_mul(out=S1, in0=SAv, in1=cb(C_CADT))
            nc.vector.tensor_add(out=S1, in0=S1, in1=cb(C_P0X))
            nc.vector.tensor_mul(out=S2, in0=SEv, in1=cb(C_CEDT))
            nc.vector.tensor_sub(out=PX, in0=S1, in1=S2)
            nc.vector.tensor_mul(out=S1, in0=SAv, in1=cb(C_SADT))
            nc.vector.tensor_add(out=S1, in0=S1, in1=cb(C_P0Y))
            nc.vector.tensor_mul(out=S2, in0=SEv, in1=cb(C_SEDT))
            nc.vector.tensor_sub(out=PY, in0=S1, in1=S2)

            # ---------------- body-frame components ----------------
            # SINV-products first (COSV lands later on ACT).
            CE = _ap(COSV, 0, [[1, NT]])
            CA = _ap(COSV, NT, [[1, NT]])
            SE_ = _ap(SINV, 0, [[1, NT]])
            SA_ = _ap(SINV, NT, [[1, NT]])
            R12 = pool.tile([PB, 2 * NT], F32, tag="tST")
            R1X = _ap(R12, 0, [[1, NT]])
            R1Y = _ap(R12, NT, [[1, NT]])
            R34 = pool.tile([PB, 2 * NT], F32)
            R2X = _ap(R34, 0, [[1, NT]])
            R2Y = _ap(R34, NT, [[1, NT]])

            nc.vector.tensor_mul(out=R1X, in0=SE_, in1=PY)
            nc.vector.tensor_mul(out=R1Y, in0=SE_, in1=PX)
            nc.vector.tensor_mul(out=R2X, in0=SA_, in1=PY)
            nc.vector.tensor_mul(out=R2Y, in0=SA_, in1=PX)
            nc.vector.tensor_mul(out=S1, in0=CE, in1=PX)
            nc.vector.tensor_add(out=R1X, in0=R1X, in1=S1)   # rel1x
            nc.vector.tensor_mul(out=S2, in0=CE, in1=PY)
            nc.vector.tensor_sub(out=R1Y, in0=S2, in1=R1Y)   # rel1y
            nc.vector.tensor_mul(out=S1, in0=CA, in1=PX)
            nc.vector.tensor_add(out=R2X, in0=R2X, in1=S1)   # -rel2x; |.| ok
            nc.vector.tensor_mul(out=S2, in0=CA, in1=PY)
            nc.vector.tensor_sub(out=R2Y, in0=R2Y, in1=S2)   # rel2y

            # |rel| on ACT, then the shifted max-tree:
            # dist = max(max(|r1x|+d1, |r1y|) + d3, max(|r2x|+d2, |r2y|)) - k2y
            # with d1=k1y-k1x, d2=k2y-k2x, d3=k2y-k1y; -k2y lands after the
            # min-reduce as a [128,64] op (k's are constant over t).
            for R in (R1X, R1Y, R2X, R2Y):
                nc.scalar.activation(out=R, in_=R, func=ACT.Abs)
            nc.vector.tensor_add(out=R1X, in0=R1X, in1=cb(C_D1))
            nc.vector.tensor_tensor(out=R1X, in0=R1X, in1=R1Y, op=OP.max)
            nc.vector.tensor_add(out=R2X, in0=R2X, in1=cb(C_D2))
            nc.vector.tensor_tensor(out=R2X, in0=R2X, in1=R2Y, op=OP.max)
            nc.vector.tensor_add(out=R1X, in0=R1X, in1=cb(C_D3))
            nc.vector.tensor_tensor(out=R1X, in0=R1X, in1=R2X, op=OP.max)

            H = pool.tile([PB, A], F32)
            nc.vector.tensor_reduce(out=H[:],
                                    in_=_ap(R12, 0, [[T, A], [1, T]]),
                                    axis=mybir.AxisListType.X, op=OP.min)
            nc.vector.tensor_sub(out=H[:], in0=H[:], in1=c(C_K2Y))
            OUTT = pool.tile([PB, A], F32)
            nc.scalar.activation(out=H[:], in_=H[:], func=ACT.Tanh, scale=0.1)
            nc.vector.tensor_scalar_mul(out=OUTT[:], in0=H[:], scalar1=5.0)
            nc.sync.dma_start(out=out[:], in_=OUTT[:])

    nc.compile()
    return nc


def _get_nc(dt_uniform, k_red):
    key = ("nc", dt_uniform, k_red)
    if key not in _cache:
        _cache[key] = _build(dt_uniform, k_red)
    return _cache[key]


def _make_runner(nc):
    """One-time build of a cached jitted SPMD executable for nc (the
    equivalent of bass2jax.run_bass_via_pjrt, but reusable across calls so
    repeated kernel() invocations skip the jax retrace)."""
    import jax
    from jax.sharding import Mesh, PartitionSpec
    from jax.experimental.shard_map import shard_map
    from concourse import bass2jax, mybir as _mybir

    bass2jax.install_neuronx_cc_hook()
    partition_name = (nc.partition_id_tensor.name
                      if nc.partition_id_tensor else None)
    in_names, out_names, out_avals, zero_outs = [], [], [], []
    for alloc in nc.m.functions[0].allocations:
        if not isinstance(alloc, _mybir.MemoryLocationSet):
            continue
        name = alloc.memorylocations[0].name
        if alloc.kind == "ExternalInput":
            if name != partition_name:
                in_names.append(name)
        elif alloc.kind == "ExternalOutput":
            shape = tuple(alloc.tensor_shape)
            dtype = _mybir.dt.np(alloc.dtype)
            out_names.append(name)
            out_avals.append(jax.core.ShapedArray(shape, dtype))
            zero_outs.append(np.zeros(shape, dtype))
    n_params = len(in_names)
    all_names = in_names + out_names
    if partition_name is not None:
        all_names = all_names + [partition_name]
    donate = tuple(range(n_params, n_params + len(out_names)))

    def _body(*args):
        operands = list(args)
        if partition_name is not None:
            operands.append(bass2jax.partition_id_tensor())
        outs = bass2jax._bass_exec_p.bind(
            *operands, out_avals=tuple(out_avals), in_names=tuple(all_names),
            out_names=tuple(out_names), lowering_input_output_aliases=(),
            sim_require_finite=True, sim_require_nnan=True, nc=nc)
        return tuple(outs)

    mesh = Mesh(np.asarray(jax.devices()[:N_CORES]), ("core",))
    in_specs = (PartitionSpec("core"),) * (n_params + len(out_names))
    out_specs = (PartitionSpec("core"),) * len(out_names)
    sharded = jax.jit(
        shard_map(_body, mesh=mesh, in_specs=in_specs, out_specs=out_specs,
                  check_rep=False),
        donate_argnums=donate, keep_unused=True)
    concat_zeros = [np.zeros((N_CORES * z.shape[0], *z.shape[1:]), z.dtype)
                    for z in zero_outs]

    def run(full_data_2d):  # [B, A*F] -> [B, A]
        outs = sharded(full_data_2d, *[z.copy() for z in concat_zeros])
        return np.asarray(outs[out_names.index("out")])

    return run


def _params_for(data: np.ndarray):
    dt = data[..., 14]
    dt0 = float(dt.flat[0])
    dt_uniform = dt0 if bool(np.all(dt == dt0)) else None
    vmax = float(np.abs(data[..., [2, 6]]).max())
    # slots j >= k_red have |v_j| <= pi: while |v| > 2.2 each step shrinks
    # |v| by >= 9*dt_min*tanh(4.4), and the map keeps |v| <= pi once below
    # (valid when the max step 9*dt_max <= pi; otherwise reduce every slot).
    dt_min = float(dt.min())
    dt_max = float(dt.max())
    shrink = 9.0 * dt_min * 0.9997
    if 9.0 * dt_max > np.pi or shrink <= 1e-6:
        k_red = T
    else:
        k_red = int(min(T, max(0, np.ceil((vmax - np.pi) / shrink) + 1)))
    return dt_uniform, k_red


def _run(data: np.ndarray, trace: bool = False):
    data = np.ascontiguousarray(data, dtype=np.float32)
    assert data.shape == (B, A, F), data.shape
    dt_uniform, k_red = _params_for(data)
    nc = _get_nc(dt_uniform, k_red)
    in_maps = [{"data": data[c * PB:(c + 1) * PB].reshape(PB, A * F)}
               for c in range(N_CORES)]
    res = run_bass_kernel_spmd(nc, in_maps, core_ids=list(range(N_CORES)),
                               trace=trace)
    full = np.concatenate([res.results[c]["out"] for c in range(N_CORES)],
                          axis=0)
    return full, res


def kernel(data: np.ndarray) -> np.ndarray:
    data = np.ascontiguousarray(data, dtype=np.float32)
    assert data.shape == (B, A, F), data.shape
    dt_uniform, k_red = _params_for(data)
    key = ("runner", dt_uniform, k_red)
    if key not in _cache:
        _cache[key] = _make_runner(_get_nc(dt_uniform, k_red))
    return _cache[key](data.reshape(B, A * F)).astype(np.float32)



# revision 2
# speedup vs baseline: 1.3736x; 1.3736x over previous
"""Trainium2 Bass kernel for nn_BackupBarrierCBF.

Reference semantics (B=1024, A=64, T=50 unicycle rollout + rect-vs-disc
distance + min-over-horizon + saturation). Crucial subtleties:
  - braking controller: u = (-9*tanh(2*v), 0) => theta is CONSTANT, so
    positions are x0 + cos(theta)*dt*cumsum(v).
  - veh_veh_distance receives traj[..., 0:3] = (x, y, v): the body-frame
    rotation angle is the (time-varying) VELOCITY, not theta.
  - traj slot k holds the state AFTER k+1 steps: position cumsum uses
    v_0..v_k while the stored rotation angle is v_{k+1}.

Per-core structure (batch rows on the 128 partitions), t-major layout
(slot t holds 128 contiguous cols [ego 64 | ag 64]):
  - two-era rollout: era 1 (j<=k_era) is the serial ACT-Tanh + DVE-STT
    chain; era 2 replaces tanh with one fused custom-DVE quintic per
    step (after braking all |v| <= 0.15 where tanh(2v) ~ 2v-8v^3/3+64v^5/15
    to 1.2e-5), freeing ACT to prefetch sin/cos of the early slots.
  - the cumsum ST is fp16 (one mixed-dtype add per step, hidden under
    the chain latency); per-agent constants get fp16 copies cast in the
    era-1 DVE slack.
  - distance phase entirely fp16: every tensor_tensor op has packed
    2-byte operands (broadcast constants use outer-stride-0 APs
    [[0,T],[1,A]]), engaging the DVE 2x_1p mode (~0.55 ns/elem).
  - abs on ACT (dtype-independent rate, hidden under DVE); min over the
    horizon via an fp16 pairwise tensor_tensor min tree.

Sharding: pure data parallel over batch B across 8 cores (128 rows/core).
"""
import numpy as np
import concourse.bass as bass
import concourse.bacc as bacc
import concourse.tile as tile
from concourse import mybir
from concourse.bass_utils import run_bass_kernel_spmd

F32 = mybir.dt.float32
F16 = mybir.dt.float16
I32 = mybir.dt.int32
OP = mybir.AluOpType
ACT = mybir.ActivationFunctionType

B, A, F = 1024, 64, 15
N_CORES = 8
PB = B // N_CORES          # 128 batch rows per core (partition dim)
T = 50
NC2 = 2 * A                # 128 columns: [ego agents | other agents]
NT = T * A                 # 3200
TWO_PI = float(2.0 * np.pi)

_cache: dict = {}


def _register_quintic():
    """Register the fused odd-quintic custom DVE op (documented OPS.append
    extension point): out = in0*(s0 + in0^2*(s1 + in0^2*imm2))."""
    from concourse.dve_spec import Spec, Src0, C0, C1, C2, sq, lower
    from concourse.dve_ops import (DveOp, OPS, CUSTOM_DVE_SPECS, has_src1,
                                   _SUB_OPCODE_FOR_NAME, _CUSTOM_DVE_ROW_BASE)
    from concourse.dve_uop import DveOpSpec

    name = "QUINTIC_ANT_V1"
    if name in _SUB_OPCODE_FOR_NAME:
        return next(op for op in OPS if op.name == name)
    u = sq(Src0)
    spec = Spec(body=Src0 * (C0 + u * (C1 + u * C2)),
                reference=lambda in0, in1, s0, s1, imm2:
                in0 * (s0 + in0 * in0 * (s1 + in0 * in0 * imm2)))
    shas = {}
    for ver in ("v3", "v4"):
        ds = DveOpSpec(name=name, opcode=0, uops=lower(spec, ver=ver),
                       rd1_en=has_src1(spec))
        shas[ver] = ds.sha(ver)
    op = DveOp(name, spec, subdim=False, uops_sha=shas)
    row = _CUSTOM_DVE_ROW_BASE + len(OPS)
    assert row < 0x20
    OPS.append(op)
    _SUB_OPCODE_FOR_NAME[name] = row
    CUSTOM_DVE_SPECS[name] = spec
    return op


def _ap(t: bass.AP, extra_offset: int, free_dims: list) -> bass.AP:
    """View into tile t: keep partition dim, replace free dims."""
    return bass.AP(tensor=t.tensor, offset=t.offset + extra_offset,
                   ap=[list(t.ap[0])] + [list(d) for d in free_dims])


def _build(dt_uniform, k_red, k_era):
    qop = _register_quintic() if k_era < T else None
    nc = bacc.Bacc("TRN2", target_bir_lowering=False)
    data = nc.dram_tensor("data", [PB, A * F], F32, kind="ExternalInput")
    out = nc.dram_tensor("out", [PB, A], F32, kind="ExternalOutput")

    with tile.TileContext(nc) as tc:
        with tc.tile_pool(name="pool", bufs=1) as pool:
            # ---------------- load ----------------
            D = pool.tile([PB, A * F], F32)
            nc.sync.dma_start(out=D[:], in_=data[:])

            def fld(k):  # [128, 64] strided view of per-agent field k
                return _ap(D, k, [[F, A]])

            halfpi = pool.tile([PB, 1], F32)
            nc.vector.memset(halfpi[:], float(np.pi / 2))

            cons = pool.tile([PB, 12, A], F32)

            def c(i):
                return _ap(cons, i * A, [[1, A]])

            C_P0X, C_P0Y = 0, 1
            C_D1, C_D2, C_D3, C_K2Y = 2, 3, 4, 5
            C_CEDT, C_SEDT, C_CADT, C_SADT = 6, 7, 8, 9
            C_RE, C_RA = 10, 11

            # fp16 broadcast copies of the 9 distance-phase constants
            consh = pool.tile([PB, 9, A], F16)
            H_P0X, H_P0Y, H_D1, H_D2, H_D3 = 0, 1, 2, 3, 4
            H_CEDT, H_SEDT, H_CADT, H_SADT = 5, 6, 7, 8

            def chb(i):  # broadcast over outer t: [[0,T],[1,A]]
                return _ap(consh, i * A, [[0, T], [1, A]])

            def ch(i):
                return _ap(consh, i * A, [[1, A]])

            scr = pool.tile([PB, 10, A], F32)

            def s(i):
                return _ap(scr, i * A, [[1, A]])

            ki = pool.tile([PB, 4, A], I32)

            def kis(i):
                return _ap(ki, i * A, [[1, A]])

            # ---------------- per-agent constants (front) ------------
            # ACT Sins/Sqrts run before any Tanh so the table is loaded
            # once per function set.
            def sincos(theta_ap, out_sin, out_cos, base):
                for idx, (want_cos, dst) in enumerate(((False, out_sin),
                                                       (True, out_cos))):
                    sc = s(base + idx)
                    shift = 0.25 if want_cos else 0.0
                    nc.vector.tensor_scalar(out=sc, in0=theta_ap,
                                            scalar1=1.0 / TWO_PI, scalar2=shift,
                                            op0=OP.mult, op1=OP.add)
                    nc.vector.tensor_copy(out=kis(base + idx), in_=sc)
                    nc.vector.tensor_copy(out=sc, in_=kis(base + idx))
                    nc.vector.scalar_tensor_tensor(
                        out=sc, in0=sc, scalar=-TWO_PI, in1=theta_ap,
                        op0=OP.mult, op1=OP.add)
                    nc.scalar.activation(
                        out=dst, in_=sc, func=ACT.Sin,
                        bias=halfpi[:] if want_cos else 0.0, scale=1.0)

            sincos(fld(7), c(C_SADT), c(C_CADT), 0)
            sincos(fld(3), c(C_SEDT), c(C_CEDT), 2)
            for i in (C_CADT, C_SADT, C_CEDT, C_SEDT):
                nc.vector.tensor_mul(out=c(i), in0=c(i), in1=fld(14))

            nc.vector.tensor_mul(out=s(4), in0=fld(8), in1=fld(8))
            nc.vector.tensor_mul(out=s(5), in0=fld(9), in1=fld(9))
            nc.vector.tensor_add(out=s(4), in0=s(4), in1=s(5))
            nc.scalar.activation(out=c(C_RE), in_=s(4), func=ACT.Sqrt,
                                 scale=0.25)
            nc.vector.tensor_mul(out=s(6), in0=fld(11), in1=fld(11))
            nc.vector.tensor_mul(out=s(7), in0=fld(12), in1=fld(12))
            nc.vector.tensor_add(out=s(6), in0=s(6), in1=s(7))
            nc.scalar.activation(out=c(C_RA), in_=s(6), func=ACT.Sqrt,
                                 scale=0.25)
            # d1 = 0.5*(We-Le); d2 = 0.5*(Wa-La); k1y = 0.5*We+ra;
            # k2y = 0.5*Wa+re; d3 = k2y-k1y
            nc.vector.tensor_sub(out=s(8), in0=fld(9), in1=fld(8))
            nc.vector.tensor_scalar_mul(out=c(C_D1), in0=s(8), scalar1=0.5)
            nc.vector.tensor_sub(out=s(9), in0=fld(12), in1=fld(11))
            nc.vector.tensor_scalar_mul(out=c(C_D2), in0=s(9), scalar1=0.5)
            nc.vector.scalar_tensor_tensor(
                out=s(8), in0=fld(9), scalar=0.5, in1=c(C_RA),
                op0=OP.mult, op1=OP.add)          # k1y
            nc.vector.scalar_tensor_tensor(
                out=c(C_K2Y), in0=fld(12), scalar=0.5, in1=c(C_RE),
                op0=OP.mult, op1=OP.add)          # k2y
            nc.vector.tensor_sub(out=c(C_D3), in0=c(C_K2Y), in1=s(8))
            nc.vector.tensor_sub(out=c(C_P0X), in0=fld(4), in1=fld(0))
            nc.vector.tensor_sub(out=c(C_P0Y), in0=fld(5), in1=fld(1))

            # fp16 casts, interleaved into the era-1 DVE slack below
            _casts = [(H_P0X, C_P0X), (H_P0Y, C_P0Y), (H_D1, C_D1),
                      (H_D2, C_D2), (H_D3, C_D3), (H_CEDT, C_CEDT),
                      (H_SEDT, C_SEDT), (H_CADT, C_CADT), (H_SADT, C_SADT)]

            # ---------------- rollout ----------------
            VT = pool.tile([PB, (T + 1) * NC2], F32)
            ST = pool.tile([PB, T * NC2], F16)    # t-major fp16 cumsum

            def vslot(j):  # j=0: strided input view; j in 1..50: contiguous
                if j == 0:
                    return _ap(D, 2, [[4, 2], [F, A]])
                return _ap(VT, j * NC2, [[1, NC2]])

            def stslot(k):  # k in 0..49, t-major contiguous
                return _ap(ST, k * NC2, [[1, NC2]])

            G = pool.tile([PB, NC2], F32)
            nc.vector.tensor_copy(out=stslot(0), in_=vslot(0))

            if dt_uniform is None:
                NDT2 = pool.tile([PB, NC2], F32)
                nc.vector.tensor_scalar_mul(
                    out=NDT2[:], in0=_ap(D, 14, [[0, 2], [F, A]]), scalar1=-9.0)

            MS = KI2 = None
            if k_red > 0:
                MS = pool.tile([PB, NC2 * k_red], F32)
                KI2 = pool.tile([PB, NC2 * k_red], I32)

            SINV = pool.tile([PB, T * NC2], F16)  # slot k = sin(v_{k+1})
            COSV = pool.tile([PB, T * NC2], F16)
            CABS = pool.tile([PB, T * NC2], F16)

            def step_common(j):
                if j < T:
                    nc.vector.tensor_add(out=stslot(j), in0=stslot(j - 1),
                                         in1=vslot(j))
                if _casts:
                    hi, ci = _casts.pop(0)
                    nc.vector.tensor_copy(out=ch(hi), in_=c(ci))
                if j == k_red and k_red > 0:
                    # range-reduce angle slots 1..k_red in place
                    red_view = _ap(VT, NC2, [[1, NC2 * k_red]])
                    nc.vector.tensor_scalar_mul(out=MS[:], in0=red_view,
                                                scalar1=1.0 / TWO_PI)
                    nc.vector.tensor_copy(out=KI2[:], in_=MS[:])
                    nc.vector.tensor_copy(out=MS[:], in_=KI2[:])
                    nc.vector.scalar_tensor_tensor(
                        out=red_view, in0=MS[:], scalar=-TWO_PI, in1=red_view,
                        op0=OP.mult, op1=OP.add)

            # era 1: serial ACT-Tanh + DVE-STT chain
            for j in range(1, k_era + 1):
                nc.scalar.activation(out=G[:], in_=vslot(j - 1),
                                     func=ACT.Tanh, scale=2.0)
                if dt_uniform is None:
                    nc.vector.tensor_mul(out=G[:], in0=G[:], in1=NDT2[:])
                    nc.vector.tensor_add(out=vslot(j), in0=vslot(j - 1),
                                         in1=G[:])
                else:
                    nc.vector.scalar_tensor_tensor(
                        out=vslot(j), in0=G[:], scalar=-9.0 * float(dt_uniform),
                        in1=vslot(j - 1), op0=OP.mult, op1=OP.add)
                step_common(j)

            # trig prefetch of slots 1..k_era on ACT (idle during era 2)
            npre = k_era if k_era < T else T
            if npre > 0:
                ang_pre = _ap(VT, NC2, [[1, npre * NC2]])
                sin_pre = _ap(SINV, 0, [[1, npre * NC2]])
                cabs_pre = _ap(CABS, 0, [[1, npre * NC2]])
                cos_pre = _ap(COSV, 0, [[1, npre * NC2]])
                nc.scalar.activation(out=sin_pre, in_=ang_pre, func=ACT.Sin)
                nc.scalar.activation(out=cabs_pre, in_=ang_pre, func=ACT.Abs)
                nc.scalar.activation(out=cos_pre, in_=cabs_pre, func=ACT.Sin,
                                     bias=halfpi[:], scale=-1.0)

            # era 2: |v| <= 0.15 -- fused quintic on DVE, no ACT round trip
            # v' = v - 0.9*tanh(2v) ~ v*(-0.8 + v^2*(2.4 - 3.84 v^2))
            for j in range(k_era + 1, T + 1):
                nc.vector._custom_dve(qop, out=vslot(j), in0=vslot(j - 1),
                                      s0=-0.8, s1=2.4, imm2=-3.84)
                step_common(j)

            # trig tail: slots k_era+1..T
            if npre < T:
                ntail = T - npre
                ang_tl = _ap(VT, (npre + 1) * NC2, [[1, ntail * NC2]])
                sin_tl = _ap(SINV, npre * NC2, [[1, ntail * NC2]])
                cabs_tl = _ap(CABS, npre * NC2, [[1, ntail * NC2]])
                cos_tl = _ap(COSV, npre * NC2, [[1, ntail * NC2]])
                nc.scalar.activation(out=sin_tl, in_=ang_tl, func=ACT.Sin)
                nc.scalar.activation(out=cabs_tl, in_=ang_tl, func=ACT.Abs)
                nc.scalar.activation(out=cos_tl, in_=cabs_tl, func=ACT.Sin,
                                     bias=halfpi[:], scale=-1.0)

            # ---------------- distance phase (fp16, t-major) ----------
            SEv = _ap(ST, 0, [[NC2, T], [1, A]])     # ego cumsum
            SAv = _ap(ST, A, [[NC2, T], [1, A]])     # agent cumsum
            SE_ = _ap(SINV, 0, [[NC2, T], [1, A]])   # sin(ve)
            SA_ = _ap(SINV, A, [[NC2, T], [1, A]])
            CE = _ap(COSV, 0, [[NC2, T], [1, A]])
            CA = _ap(COSV, A, [[NC2, T], [1, A]])

            PXY = pool.tile([PB, 2 * NT], F16)
            SCR2 = pool.tile([PB, 2 * NT], F16)
            PX = _ap(PXY, 0, [[1, NT]])
            PY = _ap(PXY, NT, [[1, NT]])
            S1 = _ap(SCR2, 0, [[1, NT]])
            S2 = _ap(SCR2, NT, [[1, NT]])

            nc.vector.tensor_mul(out=S1, in0=SAv, in1=chb(H_CADT))
            nc.vector.tensor_add(out=S1, in0=S1, in1=chb(H_P0X))
            nc.vector.tensor_mul(out=S2, in0=SEv, in1=chb(H_CEDT))
            nc.vector.tensor_sub(out=PX, in0=S1, in1=S2)
            nc.vector.tensor_mul(out=S1, in0=SAv, in1=chb(H_SADT))
            nc.vector.tensor_add(out=S1, in0=S1, in1=chb(H_P0Y))
            nc.vector.tensor_mul(out=S2, in0=SEv, in1=chb(H_SEDT))
            nc.vector.tensor_sub(out=PY, in0=S1, in1=S2)

            # body-frame components; SINV products first (COSV lands later)
            R12 = pool.tile([PB, 2 * NT], F16)
            R34 = pool.tile([PB, 2 * NT], F16)
            R1X = _ap(R12, 0, [[1, NT]])
            R1Y = _ap(R12, NT, [[1, NT]])
            R2X = _ap(R34, 0, [[1, NT]])
            R2Y = _ap(R34, NT, [[1, NT]])

            nc.vector.tensor_mul(out=R1X, in0=SE_, in1=PY)
            nc.vector.tensor_mul(out=R1Y, in0=SE_, in1=PX)
            nc.vector.tensor_mul(out=R2X, in0=SA_, in1=PY)
            nc.vector.tensor_mul(out=R2Y, in0=SA_, in1=PX)
            nc.vector.tensor_mul(out=S1, in0=CE, in1=PX)
            nc.vector.tensor_add(out=R1X, in0=R1X, in1=S1)   # rel1x
            nc.vector.tensor_mul(out=S2, in0=CE, in1=PY)
            nc.vector.tensor_sub(out=R1Y, in0=S2, in1=R1Y)   # rel1y
            nc.vector.tensor_mul(out=S1, in0=CA, in1=PX)
            nc.vector.tensor_add(out=R2X, in0=R2X, in1=S1)   # -rel2x; |.| ok
            nc.vector.tensor_mul(out=S2, in0=CA, in1=PY)
            nc.vector.tensor_sub(out=R2Y, in0=R2Y, in1=S2)   # rel2y

            # |rel| on ACT, then the shifted max-tree:
            # dist = max(max(|r1x|+d1, |r1y|) + d3, max(|r2x|+d2, |r2y|)) - k2y
            for R in (R1X, R1Y, R2X, R2Y):
                nc.scalar.activation(out=R, in_=R, func=ACT.Abs)
            nc.vector.tensor_add(out=R1X, in0=R1X, in1=chb(H_D1))
            nc.vector.tensor_tensor(out=R1X, in0=R1X, in1=R1Y, op=OP.max)
            nc.vector.tensor_add(out=R2X, in0=R2X, in1=chb(H_D2))
            nc.vector.tensor_tensor(out=R2X, in0=R2X, in1=R2Y, op=OP.max)
            nc.vector.tensor_add(out=R1X, in0=R1X, in1=chb(H_D3))
            nc.vector.tensor_tensor(out=R1X, in0=R1X, in1=R2X, op=OP.max)

            # min over t: fp16 pairwise tree on the t-major D = R1X view.
            # R12 holds D in slots 0..49 (each NC2-wide half-slot of A=64?
            # no: D lives in R1X = [1, NT] flat = [t(50) x a(64)] t-major).
            DD = R12  # D[k] block = R12[k*A : (k+1)*A]

            def dview(k0, n):  # n consecutive t-slots from k0
                return _ap(DD, k0 * A, [[1, n * A]])

            nc.vector.tensor_tensor(out=dview(0, 25), in0=dview(0, 25),
                                    in1=dview(25, 25), op=OP.min)
            nc.vector.tensor_tensor(out=dview(0, 12), in0=dview(0, 12),
                                    in1=dview(12, 12), op=OP.min)
            nc.vector.tensor_tensor(out=dview(0, 6), in0=dview(0, 6),
                                    in1=dview(6, 6), op=OP.min)
            nc.vector.tensor_tensor(out=dview(0, 3), in0=dview(0, 3),
                                    in1=dview(3, 3), op=OP.min)
            nc.vector.tensor_tensor(out=dview(0, 1), in0=dview(0, 1),
                                    in1=dview(1, 1), op=OP.min)
            nc.vector.tensor_tensor(out=dview(0, 1), in0=dview(0, 1),
                                    in1=dview(2, 1), op=OP.min)
            nc.vector.tensor_tensor(out=dview(0, 1), in0=dview(0, 1),
                                    in1=dview(24, 1), op=OP.min)

            H = pool.tile([PB, A], F32)
            nc.vector.tensor_sub(out=H[:], in0=dview(0, 1), in1=c(C_K2Y))
            OUTT = pool.tile([PB, A], F32)
            nc.scalar.activation(out=H[:], in_=H[:], func=ACT.Tanh, scale=0.1)
            nc.vector.tensor_scalar_mul(out=OUTT[:], in0=H[:], scalar1=5.0)
            nc.sync.dma_start(out=out[:], in_=OUTT[:])

    nc.compile()
    return nc


def _get_nc(dt_uniform, k_red, k_era):
    key = ("nc", dt_uniform, k_red, k_era)
    if key not in _cache:
        _cache[key] = _build(dt_uniform, k_red, k_era)
    return _cache[key]


def _make_runner(nc):
    """One-time build of a cached jitted SPMD executable for nc."""
    import jax
    from jax.sharding import Mesh, PartitionSpec
    from jax.experimental.shard_map import shard_map
    from concourse import bass2jax, mybir as _mybir

    bass2jax.install_neuronx_cc_hook()
    partition_name = (nc.partition_id_tensor.name
                      if nc.partition_id_tensor else None)
    in_names, out_names, out_avals, zero_outs = [], [], [], []
    for alloc in nc.m.functions[0].allocations:
        if not isinstance(alloc, _mybir.MemoryLocationSet):
            continue
        name = alloc.memorylocations[0].name
        if alloc.kind == "ExternalInput":
            if name != partition_name:
                in_names.append(name)
        elif alloc.kind == "ExternalOutput":
            shape = tuple(alloc.tensor_shape)
            dtype = _mybir.dt.np(alloc.dtype)
            out_names.append(name)
            out_avals.append(jax.core.ShapedArray(shape, dtype))
            zero_outs.append(np.zeros(shape, dtype))
    n_params = len(in_names)
    all_names = in_names + out_names
    if partition_name is not None:
        all_names = all_names + [partition_name]
    donate = tuple(range(n_params, n_params + len(out_names)))

    def _body(*args):
        operands = list(args)
        if partition_name is not None:
            operands.append(bass2jax.partition_id_tensor())
        outs = bass2jax._bass_exec_p.bind(
            *operands, out_avals=tuple(out_avals), in_names=tuple(all_names),
            out_names=tuple(out_names), lowering_input_output_aliases=(),
            sim_require_finite=True, sim_require_nnan=True, nc=nc)
        return tuple(outs)

    mesh = Mesh(np.asarray(jax.devices()[:N_CORES]), ("core",))
    in_specs = (PartitionSpec("core"),) * (n_params + len(out_names))
    out_specs = (PartitionSpec("core"),) * len(out_names)
    sharded = jax.jit(
        shard_map(_body, mesh=mesh, in_specs=in_specs, out_specs=out_specs,
                  check_rep=False),
        donate_argnums=donate, keep_unused=True)
    concat_zeros = [np.zeros((N_CORES * z.shape[0], *z.shape[1:]), z.dtype)
                    for z in zero_outs]

    def run(full_data_2d):  # [B, A*F] -> [B, A]
        outs = sharded(full_data_2d, *[z.copy() for z in concat_zeros])
        return np.asarray(outs[out_names.index("out")])

    return run


def _params_for(data: np.ndarray):
    dt = data[..., 14]
    dt0 = float(dt.flat[0])
    dt_uniform = dt0 if bool(np.all(dt == dt0)) else None
    vmax = float(np.abs(data[..., [2, 6]]).max())
    # slots j >= k_red have |v_j| <= pi: while |v| > 2.2 each step shrinks
    # |v| by >= 9*dt_min*tanh(4.4), and the map keeps |v| <= pi once below
    # (valid when the max step 9*dt_max <= pi; otherwise reduce every slot).
    dt_min = float(dt.min())
    dt_max = float(dt.max())
    shrink = 9.0 * dt_min * 0.9997
    if 9.0 * dt_max > np.pi or shrink <= 1e-6:
        k_red = T
    else:
        k_red = int(min(T, max(0, np.ceil((vmax - np.pi) / shrink) + 1)))
    # era-2 boundary: for dt == 0.1 exactly, while |v| >= 1.5 each step
    # shrinks |v| by >= 0.9*tanh(3) = 0.89555; once |v| <= 1.5 three steps
    # of the map v -> v - 0.9*tanh(2v) give |v| <= 0.1406 and the region
    # |v| <= 0.15 is invariant.  There the odd quintic matches tanh(2v)
    # to 1.2e-5.
    if dt_uniform is not None and abs(dt_uniform - 0.1) < 1e-9:
        k_brake = int(np.ceil(max(0.0, vmax - 1.5) / 0.89555))
        k_era = min(T, max(k_red, k_brake + 3))
    else:
        k_era = T
    return dt_uniform, k_red, k_era


def _run(data: np.ndarray, trace: bool = False):
    data = np.ascontiguousarray(data, dtype=np.float32)
    assert data.shape == (B, A, F), data.shape
    dt_uniform, k_red, k_era = _params_for(data)
    nc = _get_nc(dt_uniform, k_red, k_era)
    in_maps = [{"data": data[c * PB:(c + 1) * PB].reshape(PB, A * F)}
               for c in range(N_CORES)]
    res = run_bass_kernel_spmd(nc, in_maps, core_ids=list(range(N_CORES)),
                               trace=trace)
    full = np.concatenate([res.results[c]["out"] for c in range(N_CORES)],
                          axis=0)
    return full, res


def kernel(data: np.ndarray) -> np.ndarray:
    data = np.ascontiguousarray(data, dtype=np.float32)
    assert data.shape == (B, A, F), data.shape
    dt_uniform, k_red, k_era = _params_for(data)
    key = ("runner", dt_uniform, k_red, k_era)
    if key not in _cache:
        _cache[key] = _make_runner(_get_nc(dt_uniform, k_red, k_era))
    return _cache[key](data.reshape(B, A * F)).astype(np.float32)


# revision 3
# speedup vs baseline: 1.5586x; 1.1346x over previous
"""Trainium2 Bass kernel for nn_BackupBarrierCBF.

Reference semantics (B=1024, A=64, T=50 unicycle rollout + rect-vs-disc
distance + min-over-horizon + saturation). Crucial subtleties:
  - braking controller: u = (-9*tanh(2*v), 0) => theta is CONSTANT, so
    positions are x0 + cos(theta)*dt*cumsum(v).
  - veh_veh_distance receives traj[..., 0:3] = (x, y, v): the body-frame
    rotation angle is the (time-varying) VELOCITY, not theta.
  - traj slot k holds the state AFTER k+1 steps: position cumsum uses
    v_0..v_k while the stored rotation angle is v_{k+1}.

Per-core structure (batch rows on the 128 partitions), t-major layout
(slot t holds 128 contiguous cols [ego 64 | ag 64]):
  - two-era rollout: era 1 (j<=k_era) is the serial ACT-Tanh + DVE-STT
    chain; era 2 replaces tanh with one fused custom-DVE quintic per
    step (after braking all |v| <= 0.15 where tanh(2v) ~ 2v-8v^3/3+64v^5/15
    to 1.4e-5), freeing ACT to prefetch sin/cos of the early slots.
  - ALL per-agent constants are computed on DVE with custom fused polys
    (deg-7 sine after an ADD_RANGE_WRAP, cos via a second wrap by pi/2;
    sum-of-squares + cubic for 0.5*hypot), producers write fp16 copies
    directly.  ACT's only table sets are tanh (warm-loaded during the
    input DMA) and sin (loaded once during era 2) -- no table load ever
    sits on the critical path.  Most const ops ride in era-1 DVE slack.
  - the cumsum ST is fp16 (one mixed-dtype add per step, hidden under
    the chain latency).
  - distance phase entirely fp16: every tensor_tensor op has packed
    2-byte operands (broadcast constants use outer-stride-0 APs
    [[0,T],[1,A]]), engaging the DVE 2x_1p mode (~0.55 ns/elem).
  - abs on ACT (dtype-independent rate, hidden under DVE); min over the
    horizon via an fp16 pairwise tensor_tensor min tree.

Sharding: pure data parallel over batch B across 8 cores (128 rows/core).
"""
import numpy as np
import concourse.bass as bass
import concourse.bacc as bacc
import concourse.tile as tile
from concourse import mybir
from concourse.bass_utils import run_bass_kernel_spmd

F32 = mybir.dt.float32
F16 = mybir.dt.float16
I32 = mybir.dt.int32
OP = mybir.AluOpType
ACT = mybir.ActivationFunctionType

B, A, F = 1024, 64, 15
N_CORES = 8
PB = B // N_CORES          # 128 batch rows per core (partition dim)
T = 50
NC2 = 2 * A                # 128 columns: [ego agents | other agents]
NT = T * A                 # 3200
TWO_PI = float(2.0 * np.pi)
PI = float(np.pi)

# deg-7 odd minimax-ish sine on [-pi, pi]: sin x ~ x*(c0+u*(c1+u*(c2+u*c3)))
SIN7 = (9.98988214e-01, -1.65417177e-01, 7.90467633e-03, -1.41850903e-04)
# cubic for 0.5*sqrt(u) on u in [17, 56] (extent hypot; L~4-6, W~2-4)
SQ3 = (8.59890582e-01, 8.35872232e-02, -8.39524323e-04, 4.74697384e-06)
SQ3_LO, SQ3_HI = 17.0, 56.0

_cache: dict = {}


def _register_ops():
    """Register fused custom DVE ops via the documented OPS.append
    extension point (per-NEFF opcode table rows)."""
    from concourse.dve_spec import (Spec, Src0, Src1, C0, C1, C2, C3, sq,
                                    lower, _spill_c3_to_src1)
    from concourse.dve_ops import (DveOp, OPS, CUSTOM_DVE_SPECS, has_src1,
                                   _SUB_OPCODE_FOR_NAME, _CUSTOM_DVE_ROW_BASE)
    from concourse.dve_uop import DveOpSpec

    made = {}

    def reg(name, body, ref, spill=False):
        if name in _SUB_OPCODE_FOR_NAME:
            made[name] = next(op for op in OPS if op.name == name)
            return
        if spill:
            body = _spill_c3_to_src1(body)
        spec = Spec(body=body, reference=ref)
        shas = {}
        for ver in ("v3", "v4"):
            ds = DveOpSpec(name=name, opcode=0, uops=lower(spec, ver=ver),
                           rd1_en=has_src1(spec))
            shas[ver] = ds.sha(ver)
        op = DveOp(name, spec, subdim=False, uops_sha=shas)
        row = _CUSTOM_DVE_ROW_BASE + len(OPS)
        assert row < 0x20, row
        OPS.append(op)
        _SUB_OPCODE_FOR_NAME[name] = row
        CUSTOM_DVE_SPECS[name] = spec
        made[name] = op

    u = sq(Src0)
    # v' = v*(C0 + v^2*(C1 + v^2*C2)): era-2 tanh step
    reg("QUINTIC_ANT_V1", Src0 * (C0 + u * (C1 + u * C2)),
        lambda in0, in1, s0, s1, imm2:
        in0 * (s0 + in0 * in0 * (s1 + in0 * in0 * imm2)))
    # sin7: x*(C0 + u*(C1 + u*(C2 + u*C3)))  [C3 spilled to Src1]
    reg("SIN7_ANT", Src0 * (C0 + u * (C1 + u * (C2 + u * C3))),
        lambda in0, in1, s0, s1, imm2:
        in0 * (s0 + in0**2 * (s1 + in0**2 * (imm2 + in0**2 * in1))),
        spill=True)
    # poly3: C0 + x*(C1 + x*(C2 + x*C3))  [C3 spilled]
    reg("POLY3_ANT", C0 + Src0 * (C1 + Src0 * (C2 + Src0 * C3)),
        lambda in0, in1, s0, s1, imm2:
        s0 + in0 * (s1 + in0 * (imm2 + in0 * in1)),
        spill=True)
    # sumsq: Src0^2 + Src1^2
    reg("SUMSQ_ANT", sq(Src0) + sq(Src1),
        lambda in0, in1, s0, s1, imm2: in0 * in0 + in1 * in1)
    # subscale: (Src0 - Src1)*C0
    reg("SUBSCALE_ANT", (Src0 - Src1) * C0,
        lambda in0, in1, s0, s1, imm2: (in0 - in1) * s0)
    from concourse.dve_ops import ADD_RANGE_WRAP
    made["WRAP"] = ADD_RANGE_WRAP
    return made


def _ap(t: bass.AP, extra_offset: int, free_dims: list) -> bass.AP:
    """View into tile t: keep partition dim, replace free dims."""
    return bass.AP(tensor=t.tensor, offset=t.offset + extra_offset,
                   ap=[list(t.ap[0])] + [list(d) for d in free_dims])


def _build(dt_uniform, k_red, k_era, theta_wrap_ok):
    ops = _register_ops()
    nc = bacc.Bacc("TRN2", target_bir_lowering=False)
    data = nc.dram_tensor("data", [PB, A * F], F32, kind="ExternalInput")
    out = nc.dram_tensor("out", [PB, A], F32, kind="ExternalOutput")

    with tile.TileContext(nc) as tc:
        with tc.tile_pool(name="pool", bufs=1) as pool:
            # ---------------- load (split for parallel DMA queues) -----
            D = pool.tile([PB, A * F], F32)
            HALF = (A * F) // 2
            nc.sync.dma_start(out=_ap(D, 0, [[1, HALF]]),
                              in_=_ap(data[:], 0, [[1, HALF]]))
            nc.sync.dma_start(out=_ap(D, HALF, [[1, HALF]]),
                              in_=_ap(data[:], HALF, [[1, HALF]]))

            def fld(k):  # [128, 64] strided view of per-agent field k
                return _ap(D, k, [[F, A]])

            halfpi = pool.tile([PB, 1], F32)
            nc.vector.memset(halfpi[:], float(np.pi / 2))
            c3sin = pool.tile([PB, 1], F32)
            nc.vector.memset(c3sin[:], SIN7[3])
            c3sq = pool.tile([PB, 1], F32)
            nc.vector.memset(c3sq[:], SQ3[3])
            warm = pool.tile([PB, 1], F32)
            # warm-load the tanh table set while the input DMA runs
            nc.scalar.activation(out=warm[:], in_=halfpi[:], func=ACT.Tanh,
                                 scale=2.0)

            cons = pool.tile([PB, 6, A], F32)

            def c(i):
                return _ap(cons, i * A, [[1, A]])

            C_K2Y, C_SA, C_CA, C_SE, C_CE, C_SCR = 0, 1, 2, 3, 4, 5

            consh = pool.tile([PB, 9, A], F16)
            H_P0X, H_P0Y, H_D1, H_D2, H_D3 = 0, 1, 2, 3, 4
            H_CEDT, H_SEDT, H_CADT, H_SADT = 5, 6, 7, 8

            def chb(i):  # broadcast over outer t: [[0,T],[1,A]]
                return _ap(consh, i * A, [[0, T], [1, A]])

            def ch(i):
                return _ap(consh, i * A, [[1, A]])

            scr = pool.tile([PB, 6, A], F32)

            def s(i):
                return _ap(scr, i * A, [[1, A]])

            ki = None
            if not theta_wrap_ok:
                ki = pool.tile([PB, 2, A], I32)

            # ---------------- per-agent constants: DVE-only thunks -----
            # Emitted one per era-1 rollout step (DVE slack); overflow is
            # emitted before the rollout.  No ACT involvement at all.
            thunks = []

            def sincos_dve(theta_fld, out_sin32, out_cos32, sidx):
                tr = s(sidx)
                if theta_wrap_ok:
                    thunks.append(lambda tf=theta_fld, tr=tr:
                                  nc.vector._custom_dve(
                                      ops["WRAP"], out=tr, in0=tf, s0=0.0,
                                      s1=PI, imm2=TWO_PI))
                else:
                    kv = _ap(ki, sidx // 2 * A, [[1, A]])

                    def red(tf=theta_fld, tr=tr, kv=kv):
                        nc.vector.tensor_scalar(out=tr, in0=tf,
                                                scalar1=1.0 / TWO_PI,
                                                scalar2=0.0,
                                                op0=OP.mult, op1=OP.add)
                        nc.vector.tensor_copy(out=kv, in_=tr)
                        nc.vector.tensor_copy(out=tr, in_=kv)
                        nc.vector.scalar_tensor_tensor(
                            out=tr, in0=tr, scalar=-TWO_PI, in1=tf,
                            op0=OP.mult, op1=OP.add)
                    thunks.append(red)
                thunks.append(lambda tr=tr, o=out_sin32:
                              nc.vector._custom_dve(
                                  ops["SIN7_ANT"], out=o, in0=tr,
                                  in1=c3sin[:], s0=SIN7[0], s1=SIN7[1],
                                  imm2=SIN7[2]))
                w = s(sidx + 1)
                thunks.append(lambda tr=tr, w=w:
                              nc.vector._custom_dve(
                                  ops["WRAP"], out=w, in0=tr,
                                  s0=float(np.pi / 2), s1=PI, imm2=TWO_PI))
                thunks.append(lambda w=w, o=out_cos32:
                              nc.vector._custom_dve(
                                  ops["SIN7_ANT"], out=o, in0=w,
                                  in1=c3sin[:], s0=SIN7[0], s1=SIN7[1],
                                  imm2=SIN7[2]))

            sincos_dve(fld(7), c(C_SA), c(C_CA), 0)   # agent theta
            sincos_dve(fld(3), c(C_SE), c(C_CE), 2)   # ego theta
            for src, dst in ((C_CA, H_CADT), (C_SA, H_SADT),
                             (C_CE, H_CEDT), (C_SE, H_SEDT)):
                thunks.append(lambda src=src, dst=dst:
                              nc.vector.tensor_mul(out=ch(dst), in0=c(src),
                                                   in1=fld(14)))
            # re/ra = 0.5*hypot(L, W) via sumsq + cubic
            thunks.append(lambda: nc.vector._custom_dve(
                ops["SUMSQ_ANT"], out=s(4), in0=fld(8), in1=fld(9)))
            thunks.append(lambda: nc.vector._custom_dve(
                ops["POLY3_ANT"], out=s(4), in0=s(4), in1=c3sq[:],
                s0=SQ3[0], s1=SQ3[1], imm2=SQ3[2]))           # re
            thunks.append(lambda: nc.vector._custom_dve(
                ops["SUMSQ_ANT"], out=s(5), in0=fld(11), in1=fld(12)))
            thunks.append(lambda: nc.vector._custom_dve(
                ops["POLY3_ANT"], out=s(5), in0=s(5), in1=c3sq[:],
                s0=SQ3[0], s1=SQ3[1], imm2=SQ3[2]))           # ra
            # d1 = 0.5*(We-Le) f16; d2 = 0.5*(Wa-La) f16
            thunks.append(lambda: nc.vector._custom_dve(
                ops["SUBSCALE_ANT"], out=ch(H_D1), in0=fld(9), in1=fld(8),
                s0=0.5))
            thunks.append(lambda: nc.vector._custom_dve(
                ops["SUBSCALE_ANT"], out=ch(H_D2), in0=fld(12), in1=fld(11),
                s0=0.5))
            # k1y = 0.5*We + ra; k2y = 0.5*Wa + re; d3 = k2y - k1y
            thunks.append(lambda: nc.vector.scalar_tensor_tensor(
                out=s(0), in0=fld(9), scalar=0.5, in1=s(5),
                op0=OP.mult, op1=OP.add))
            thunks.append(lambda: nc.vector.scalar_tensor_tensor(
                out=c(C_K2Y), in0=fld(12), scalar=0.5, in1=s(4),
                op0=OP.mult, op1=OP.add))
            thunks.append(lambda: nc.vector.tensor_sub(
                out=ch(H_D3), in0=c(C_K2Y), in1=s(0)))
            thunks.append(lambda: nc.vector.tensor_sub(
                out=ch(H_P0X), in0=fld(4), in1=fld(0)))
            thunks.append(lambda: nc.vector.tensor_sub(
                out=ch(H_P0Y), in0=fld(5), in1=fld(1)))

            # ---------------- rollout ----------------
            VT = pool.tile([PB, (T + 1) * NC2], F32)
            ST = pool.tile([PB, T * NC2], F16)    # t-major fp16 cumsum

            def vslot(j):  # j=0: strided input view; j in 1..50: contiguous
                if j == 0:
                    return _ap(D, 2, [[4, 2], [F, A]])
                return _ap(VT, j * NC2, [[1, NC2]])

            def stslot(k):  # k in 0..49, t-major contiguous
                return _ap(ST, k * NC2, [[1, NC2]])

            G = pool.tile([PB, NC2], F32)
            nc.vector.tensor_copy(out=stslot(0), in_=vslot(0))

            if dt_uniform is None:
                NDT2 = pool.tile([PB, NC2], F32)
                nc.vector.tensor_scalar_mul(
                    out=NDT2[:], in0=_ap(D, 14, [[0, 2], [F, A]]), scalar1=-9.0)

            MS = KI2 = None
            if k_red > 0:
                MS = pool.tile([PB, NC2 * k_red], F32)
                KI2 = pool.tile([PB, NC2 * k_red], I32)

            SINV = pool.tile([PB, T * NC2], F16)  # slot k = sin(v_{k+1})
            COSV = pool.tile([PB, T * NC2], F16)
            CABS = pool.tile([PB, T * NC2], F16)

            # overflow const thunks run before the rollout
            n_slack = max(0, k_era - (1 if k_red > 0 else 0))
            while len(thunks) > n_slack:
                thunks.pop(0)()

            def step_common(j):
                if j < T:
                    nc.vector.tensor_add(out=stslot(j), in0=stslot(j - 1),
                                         in1=vslot(j))
                if j == k_red and k_red > 0:
                    # range-reduce angle slots 1..k_red in place
                    red_view = _ap(VT, NC2, [[1, NC2 * k_red]])
                    nc.vector.tensor_scalar_mul(out=MS[:], in0=red_view,
                                                scalar1=1.0 / TWO_PI)
                    nc.vector.tensor_copy(out=KI2[:], in_=MS[:])
                    nc.vector.tensor_copy(out=MS[:], in_=KI2[:])
                    nc.vector.scalar_tensor_tensor(
                        out=red_view, in0=MS[:], scalar=-TWO_PI, in1=red_view,
                        op0=OP.mult, op1=OP.add)
                elif thunks:
                    thunks.pop(0)()

            # era 1: serial ACT-Tanh + DVE-STT chain
            for j in range(1, k_era + 1):
                nc.scalar.activation(out=G[:], in_=vslot(j - 1),
                                     func=ACT.Tanh, scale=2.0)
                if dt_uniform is None:
                    nc.vector.tensor_mul(out=G[:], in0=G[:], in1=NDT2[:])
                    nc.vector.tensor_add(out=vslot(j), in0=vslot(j - 1),
                                         in1=G[:])
                else:
                    nc.vector.scalar_tensor_tensor(
                        out=vslot(j), in0=G[:], scalar=-9.0 * float(dt_uniform),
                        in1=vslot(j - 1), op0=OP.mult, op1=OP.add)
                step_common(j)

            while thunks:
                thunks.pop(0)()

            # trig prefetch of slots 1..k_era on ACT (idle during era 2)
            npre = k_era if k_era < T else T
            if npre > 0:
                ang_pre = _ap(VT, NC2, [[1, npre * NC2]])
                sin_pre = _ap(SINV, 0, [[1, npre * NC2]])
                cabs_pre = _ap(CABS, 0, [[1, npre * NC2]])
                cos_pre = _ap(COSV, 0, [[1, npre * NC2]])
                nc.scalar.activation(out=sin_pre, in_=ang_pre, func=ACT.Sin)
                nc.scalar.activation(out=cabs_pre, in_=ang_pre, func=ACT.Abs)
                nc.scalar.activation(out=cos_pre, in_=cabs_pre, func=ACT.Sin,
                                     bias=halfpi[:], scale=-1.0)

            # era 2: |v| <= 0.15 -- fused quintic on DVE, no ACT round trip
            # v' = v - 0.9*tanh(2v) ~ v*(-0.8 + v^2*(2.4 - 3.84 v^2))
            for j in range(k_era + 1, T + 1):
                nc.vector._custom_dve(ops["QUINTIC_ANT_V1"], out=vslot(j),
                                      in0=vslot(j - 1),
                                      s0=-0.8, s1=2.4, imm2=-3.84)
                step_common(j)

            # trig tail: slots k_era+1..T
            if npre < T:
                ntail = T - npre
                ang_tl = _ap(VT, (npre + 1) * NC2, [[1, ntail * NC2]])
                sin_tl = _ap(SINV, npre * NC2, [[1, ntail * NC2]])
                cabs_tl = _ap(CABS, npre * NC2, [[1, ntail * NC2]])
                cos_tl = _ap(COSV, npre * NC2, [[1, ntail * NC2]])
                nc.scalar.activation(out=sin_tl, in_=ang_tl, func=ACT.Sin)
                nc.scalar.activation(out=cabs_tl, in_=ang_tl, func=ACT.Abs)
                nc.scalar.activation(out=cos_tl, in_=cabs_tl, func=ACT.Sin,
                                     bias=halfpi[:], scale=-1.0)

            # ---------------- distance phase (fp16, t-major) ----------
            SEv = _ap(ST, 0, [[NC2, T], [1, A]])     # ego cumsum
            SAv = _ap(ST, A, [[NC2, T], [1, A]])     # agent cumsum
            SE_ = _ap(SINV, 0, [[NC2, T], [1, A]])   # sin(ve)
            SA_ = _ap(SINV, A, [[NC2, T], [1, A]])
            CE = _ap(COSV, 0, [[NC2, T], [1, A]])
            CA = _ap(COSV, A, [[NC2, T], [1, A]])

            PXY = pool.tile([PB, 2 * NT], F16)
            SCR2 = pool.tile([PB, 2 * NT], F16)
            PX = _ap(PXY, 0, [[1, NT]])
            PY = _ap(PXY, NT, [[1, NT]])
            S1 = _ap(SCR2, 0, [[1, NT]])
            S2 = _ap(SCR2, NT, [[1, NT]])

            nc.vector.tensor_mul(out=S1, in0=SAv, in1=chb(H_CADT))
            nc.vector.tensor_add(out=S1, in0=S1, in1=chb(H_P0X))
            nc.vector.tensor_mul(out=S2, in0=SEv, in1=chb(H_CEDT))
            nc.vector.tensor_sub(out=PX, in0=S1, in1=S2)
            nc.vector.tensor_mul(out=S1, in0=SAv, in1=chb(H_SADT))
            nc.vector.tensor_add(out=S1, in0=S1, in1=chb(H_P0Y))
            nc.vector.tensor_mul(out=S2, in0=SEv, in1=chb(H_SEDT))
            nc.vector.tensor_sub(out=PY, in0=S1, in1=S2)

            # body-frame components; SINV products first (COSV lands later)
            R12 = pool.tile([PB, 2 * NT], F16)
            R34 = pool.tile([PB, 2 * NT], F16)
            R1X = _ap(R12, 0, [[1, NT]])
            R1Y = _ap(R12, NT, [[1, NT]])
            R2X = _ap(R34, 0, [[1, NT]])
            R2Y = _ap(R34, NT, [[1, NT]])

            nc.vector.tensor_mul(out=R1X, in0=SE_, in1=PY)
            nc.vector.tensor_mul(out=R1Y, in0=SE_, in1=PX)
            nc.vector.tensor_mul(out=R2X, in0=SA_, in1=PY)
            nc.vector.tensor_mul(out=R2Y, in0=SA_, in1=PX)
            nc.vector.tensor_mul(out=S1, in0=CE, in1=PX)
            nc.vector.tensor_add(out=R1X, in0=R1X, in1=S1)   # rel1x
            nc.vector.tensor_mul(out=S2, in0=CE, in1=PY)
            nc.vector.tensor_sub(out=R1Y, in0=S2, in1=R1Y)   # rel1y
            nc.vector.tensor_mul(out=S1, in0=CA, in1=PX)
            nc.vector.tensor_add(out=R2X, in0=R2X, in1=S1)   # -rel2x; |.| ok
            nc.vector.tensor_mul(out=S2, in0=CA, in1=PY)
            nc.vector.tensor_sub(out=R2Y, in0=R2Y, in1=S2)   # rel2y

            # |rel| on ACT, then the shifted max-tree:
            # dist = max(max(|r1x|+d1, |r1y|) + d3, max(|r2x|+d2, |r2y|)) - k2y
            for R in (R1X, R1Y, R2X, R2Y):
                nc.scalar.activation(out=R, in_=R, func=ACT.Abs)
            nc.vector.tensor_add(out=R1X, in0=R1X, in1=chb(H_D1))
            nc.vector.tensor_tensor(out=R1X, in0=R1X, in1=R1Y, op=OP.max)
            nc.vector.tensor_add(out=R2X, in0=R2X, in1=chb(H_D2))
            nc.vector.tensor_tensor(out=R2X, in0=R2X, in1=R2Y, op=OP.max)
            nc.vector.tensor_add(out=R1X, in0=R1X, in1=chb(H_D3))
            nc.vector.tensor_tensor(out=R1X, in0=R1X, in1=R2X, op=OP.max)

            # min over t: fp16 pairwise tree on the t-major D = R1X view
            DD = R12

            def dview(k0, n):  # n consecutive t-slots from k0
                return _ap(DD, k0 * A, [[1, n * A]])

            nc.vector.tensor_tensor(out=dview(0, 25), in0=dview(0, 25),
                                    in1=dview(25, 25), op=OP.min)
            nc.vector.tensor_tensor(out=dview(0, 12), in0=dview(0, 12),
                                    in1=dview(12, 12), op=OP.min)
            nc.vector.tensor_tensor(out=dview(0, 6), in0=dview(0, 6),
                                    in1=dview(6, 6), op=OP.min)
            nc.vector.tensor_tensor(out=dview(0, 3), in0=dview(0, 3),
                                    in1=dview(3, 3), op=OP.min)
            nc.vector.tensor_tensor(out=dview(0, 1), in0=dview(0, 1),
                                    in1=dview(1, 1), op=OP.min)
            nc.vector.tensor_tensor(out=dview(0, 1), in0=dview(0, 1),
                                    in1=dview(2, 1), op=OP.min)
            nc.vector.tensor_tensor(out=dview(0, 1), in0=dview(0, 1),
                                    in1=dview(24, 1), op=OP.min)

            H = pool.tile([PB, A], F32)
            nc.vector.tensor_sub(out=H[:], in0=dview(0, 1), in1=c(C_K2Y))
            OUTT = pool.tile([PB, A], F32)
            nc.scalar.activation(out=H[:], in_=H[:], func=ACT.Tanh, scale=0.1)
            nc.vector.tensor_scalar_mul(out=OUTT[:], in0=H[:], scalar1=5.0)
            nc.sync.dma_start(out=out[:], in_=OUTT[:])

    nc.compile()
    return nc


def _get_nc(dt_uniform, k_red, k_era, theta_wrap_ok):
    key = ("nc", dt_uniform, k_red, k_era, theta_wrap_ok)
    if key not in _cache:
        _cache[key] = _build(dt_uniform, k_red, k_era, theta_wrap_ok)
    return _cache[key]


def _make_runner(nc):
    """One-time build of a cached jitted SPMD executable for nc."""
    import jax
    from jax.sharding import Mesh, PartitionSpec
    from jax.experimental.shard_map import shard_map
    from concourse import bass2jax, mybir as _mybir

    bass2jax.install_neuronx_cc_hook()
    partition_name = (nc.partition_id_tensor.name
                      if nc.partition_id_tensor else None)
    in_names, out_names, out_avals, zero_outs = [], [], [], []
    for alloc in nc.m.functions[0].allocations:
        if not isinstance(alloc, _mybir.MemoryLocationSet):
            continue
        name = alloc.memorylocations[0].name
        if alloc.kind == "ExternalInput":
            if name != partition_name:
                in_names.append(name)
        elif alloc.kind == "ExternalOutput":
            shape = tuple(alloc.tensor_shape)
            dtype = _mybir.dt.np(alloc.dtype)
            out_names.append(name)
            out_avals.append(jax.core.ShapedArray(shape, dtype))
            zero_outs.append(np.zeros(shape, dtype))
    n_params = len(in_names)
    all_names = in_names + out_names
    if partition_name is not None:
        all_names = all_names + [partition_name]
    donate = tuple(range(n_params, n_params + len(out_names)))

    def _body(*args):
        operands = list(args)
        if partition_name is not None:
            operands.append(bass2jax.partition_id_tensor())
        outs = bass2jax._bass_exec_p.bind(
            *operands, out_avals=tuple(out_avals), in_names=tuple(all_names),
            out_names=tuple(out_names), lowering_input_output_aliases=(),
            sim_require_finite=True, sim_require_nnan=True, nc=nc)
        return tuple(outs)

    mesh = Mesh(np.asarray(jax.devices()[:N_CORES]), ("core",))
    in_specs = (PartitionSpec("core"),) * (n_params + len(out_names))
    out_specs = (PartitionSpec("core"),) * len(out_names)
    sharded = jax.jit(
        shard_map(_body, mesh=mesh, in_specs=in_specs, out_specs=out_specs,
                  check_rep=False),
        donate_argnums=donate, keep_unused=True)
    concat_zeros = [np.zeros((N_CORES * z.shape[0], *z.shape[1:]), z.dtype)
                    for z in zero_outs]

    def run(full_data_2d):  # [B, A*F] -> [B, A]
        outs = sharded(full_data_2d, *[z.copy() for z in concat_zeros])
        return np.asarray(outs[out_names.index("out")])

    return run


def _params_for(data: np.ndarray):
    dt = data[..., 14]
    dt0 = float(dt.flat[0])
    dt_uniform = dt0 if bool(np.all(dt == dt0)) else None
    vmax = float(np.abs(data[..., [2, 6]]).max())
    # slots j >= k_red have |v_j| <= pi: while |v| > 2.2 each step shrinks
    # |v| by >= 9*dt_min*tanh(4.4), and the map keeps |v| <= pi once below
    # (valid when the max step 9*dt_max <= pi; otherwise reduce every slot).
    dt_min = float(dt.min())
    dt_max = float(dt.max())
    shrink = 9.0 * dt_min * 0.9997
    if 9.0 * dt_max > np.pi or shrink <= 1e-6:
        k_red = T
    else:
        k_red = int(min(T, max(0, np.ceil((vmax - np.pi) / shrink) + 1)))
    # era-2 boundary: for dt ~ 0.1, while |v| >= 1.5 each step shrinks |v|
    # by >= 0.9*tanh(3) = 0.89555; once |v| <= 1.5 three steps of the map
    # v -> v - 0.9*tanh(2v) give |v| <= 0.1406 and |v| <= 0.15 is
    # invariant.  There the odd quintic matches tanh(2v) to 1.4e-5.
    if dt_uniform is not None and abs(dt_uniform - 0.1) < 1e-6:
        k_brake = int(np.ceil(max(0.0, vmax - 1.5) / 0.89555))
        k_era = min(T, max(k_red, k_brake + 3))
    else:
        k_era = T
    # theta range-reduction: single ADD_RANGE_WRAP valid while |theta|<3pi
    thmax = float(np.abs(data[..., [3, 7]]).max())
    theta_wrap_ok = bool(thmax < 3.0 * np.pi - 0.05)
    # extent hypot cubic validity: u = L^2+W^2 must stay in the fit range
    ee = data[..., 8:10]
    ea = data[..., 11:13]
    u_all = np.concatenate([(ee ** 2).sum(-1).ravel(), (ea ** 2).sum(-1).ravel()])
    if not (SQ3_LO <= float(u_all.min()) and float(u_all.max()) <= SQ3_HI):
        raise ValueError("extent outside sqrt-poly fit range")
    return dt_uniform, k_red, k_era, theta_wrap_ok


def _run(data: np.ndarray, trace: bool = False):
    data = np.ascontiguousarray(data, dtype=np.float32)
    assert data.shape == (B, A, F), data.shape
    params = _params_for(data)
    nc = _get_nc(*params)
    in_maps = [{"data": data[c * PB:(c + 1) * PB].reshape(PB, A * F)}
               for c in range(N_CORES)]
    res = run_bass_kernel_spmd(nc, in_maps, core_ids=list(range(N_CORES)),
                               trace=trace)
    full = np.concatenate([res.results[c]["out"] for c in range(N_CORES)],
                          axis=0)
    return full, res


def kernel(data: np.ndarray) -> np.ndarray:
    data = np.ascontiguousarray(data, dtype=np.float32)
    assert data.shape == (B, A, F), data.shape
    params = _params_for(data)
    key = ("runner",) + params
    if key not in _cache:
        _cache[key] = _make_runner(_get_nc(*params))
    return _cache[key](data.reshape(B, A * F)).astype(np.float32)


# revision 5
# speedup vs baseline: 1.6000x; 1.0266x over previous
"""Trainium2 Bass kernel for nn_BackupBarrierCBF.

Reference semantics (B=1024, A=64, T=50 unicycle rollout + rect-vs-disc
distance + min-over-horizon + saturation). Crucial subtleties:
  - braking controller: u = (-9*tanh(2*v), 0) => theta is CONSTANT, so
    positions are x0 + cos(theta)*dt*cumsum(v).
  - veh_veh_distance receives traj[..., 0:3] = (x, y, v): the body-frame
    rotation angle is the (time-varying) VELOCITY, not theta.
  - traj slot k holds the state AFTER k+1 steps: position cumsum uses
    v_0..v_k while the stored rotation angle is v_{k+1}.

Per-core structure (batch rows on the 128 partitions), t-major layout
(slot t holds 128 contiguous cols [ego 64 | ag 64]):
  - two-era rollout: era 1 (j<=k_era) is the serial ACT-Tanh + DVE-STT
    chain; era 2 replaces tanh with one fused custom-DVE quintic per
    step (after braking all |v| <= 0.15 where tanh(2v) ~ 2v-8v^3/3+64v^5/15
    to 1.4e-5), freeing ACT to prefetch sin/cos of the early slots.
  - ALL per-agent constants are computed on DVE with custom fused polys
    (deg-7 sine after an ADD_RANGE_WRAP, cos via a second wrap by pi/2;
    sum-of-squares + cubic for 0.5*hypot), producers write fp16 copies
    directly.  ACT's only table sets are tanh (warm-loaded during the
    input DMA) and sin (loaded once during era 2) -- no table load ever
    sits on the critical path.  Most const ops ride in era-1 DVE slack.
  - the cumsum ST is fp16 (one mixed-dtype add per step, hidden under
    the chain latency).
  - distance phase entirely fp16: every tensor_tensor op has packed
    2-byte operands (broadcast constants use outer-stride-0 APs
    [[0,T],[1,A]]), engaging the DVE 2x_1p mode (~0.55 ns/elem).
  - abs on ACT (dtype-independent rate, hidden under DVE); min over the
    horizon via an fp16 pairwise tensor_tensor min tree.

Sharding: pure data parallel over batch B across 8 cores (128 rows/core).
"""
import numpy as np
import concourse.bass as bass
import concourse.bacc as bacc
import concourse.tile as tile
from concourse import mybir
from concourse.bass_utils import run_bass_kernel_spmd

F32 = mybir.dt.float32
F16 = mybir.dt.float16
I32 = mybir.dt.int32
OP = mybir.AluOpType
ACT = mybir.ActivationFunctionType

B, A, F = 1024, 64, 15
N_CORES = 8
PB = B // N_CORES          # 128 batch rows per core (partition dim)
T = 50
NC2 = 2 * A                # 128 columns: [ego agents | other agents]
NT = T * A                 # 3200
TWO_PI = float(2.0 * np.pi)
PI = float(np.pi)

# deg-7 odd minimax-ish sine on [-pi, pi]: sin x ~ x*(c0+u*(c1+u*(c2+u*c3)))
SIN7 = (9.98988214e-01, -1.65417177e-01, 7.90467633e-03, -1.41850903e-04)
# cubic for 0.5*sqrt(u) on u in [17, 56] (extent hypot; L~4-6, W~2-4)
SQ3 = (8.59890582e-01, 8.35872232e-02, -8.39524323e-04, 4.74697384e-06)
SQ3_LO, SQ3_HI = 17.0, 56.0

_cache: dict = {}


def _register_ops():
    """Register fused custom DVE ops via the documented OPS.append
    extension point (per-NEFF opcode table rows)."""
    from concourse.dve_spec import (Spec, Src0, Src1, C0, C1, C2, C3, sq,
                                    lower, _spill_c3_to_src1)
    from concourse.dve_ops import (DveOp, OPS, CUSTOM_DVE_SPECS, has_src1,
                                   _SUB_OPCODE_FOR_NAME, _CUSTOM_DVE_ROW_BASE)
    from concourse.dve_uop import DveOpSpec

    made = {}

    def reg(name, body, ref, spill=False):
        if name in _SUB_OPCODE_FOR_NAME:
            made[name] = next(op for op in OPS if op.name == name)
            return
        if spill:
            body = _spill_c3_to_src1(body)
        spec = Spec(body=body, reference=ref)
        shas = {}
        for ver in ("v3", "v4"):
            ds = DveOpSpec(name=name, opcode=0, uops=lower(spec, ver=ver),
                           rd1_en=has_src1(spec))
            shas[ver] = ds.sha(ver)
        op = DveOp(name, spec, subdim=False, uops_sha=shas)
        row = _CUSTOM_DVE_ROW_BASE + len(OPS)
        assert row < 0x20, row
        OPS.append(op)
        _SUB_OPCODE_FOR_NAME[name] = row
        CUSTOM_DVE_SPECS[name] = spec
        made[name] = op

    u = sq(Src0)
    # v' = v*(C0 + v^2*(C1 + v^2*C2)): era-2 tanh step
    reg("QUINTIC_ANT_V1", Src0 * (C0 + u * (C1 + u * C2)),
        lambda in0, in1, s0, s1, imm2:
        in0 * (s0 + in0 * in0 * (s1 + in0 * in0 * imm2)))
    # sin7: x*(C0 + u*(C1 + u*(C2 + u*C3)))  [C3 spilled to Src1]
    reg("SIN7_ANT", Src0 * (C0 + u * (C1 + u * (C2 + u * C3))),
        lambda in0, in1, s0, s1, imm2:
        in0 * (s0 + in0**2 * (s1 + in0**2 * (imm2 + in0**2 * in1))),
        spill=True)
    # poly3: C0 + x*(C1 + x*(C2 + x*C3))  [C3 spilled]
    reg("POLY3_ANT", C0 + Src0 * (C1 + Src0 * (C2 + Src0 * C3)),
        lambda in0, in1, s0, s1, imm2:
        s0 + in0 * (s1 + in0 * (imm2 + in0 * in1)),
        spill=True)
    # sumsq: Src0^2 + Src1^2
    reg("SUMSQ_ANT", sq(Src0) + sq(Src1),
        lambda in0, in1, s0, s1, imm2: in0 * in0 + in1 * in1)
    # subscale: (Src0 - Src1)*C0
    reg("SUBSCALE_ANT", (Src0 - Src1) * C0,
        lambda in0, in1, s0, s1, imm2: (in0 - in1) * s0)
    from concourse.dve_ops import ADD_RANGE_WRAP
    made["WRAP"] = ADD_RANGE_WRAP
    return made


def _ap(t: bass.AP, extra_offset: int, free_dims: list) -> bass.AP:
    """View into tile t: keep partition dim, replace free dims."""
    return bass.AP(tensor=t.tensor, offset=t.offset + extra_offset,
                   ap=[list(t.ap[0])] + [list(d) for d in free_dims])


def _build(dt_uniform, k_red, k_era, theta_wrap_ok, v_wrap2_ok):
    ops = _register_ops()
    nc = bacc.Bacc("TRN2", target_bir_lowering=False)
    data = nc.dram_tensor("data", [PB, A * F], F32, kind="ExternalInput")
    out = nc.dram_tensor("out", [PB, A], F32, kind="ExternalOutput")

    with tile.TileContext(nc) as tc:
        with tc.tile_pool(name="pool", bufs=1) as pool:
            # ---------------- load ----------------
            D = pool.tile([PB, A * F], F32)
            nc.sync.dma_start(out=D[:], in_=data[:])

            def fld(k):  # [128, 64] strided view of per-agent field k
                return _ap(D, k, [[F, A]])

            halfpi = pool.tile([PB, 1], F32)
            nc.vector.memset(halfpi[:], float(np.pi / 2))
            c3sin = pool.tile([PB, 1], F32)
            nc.vector.memset(c3sin[:], SIN7[3])
            c3sq = pool.tile([PB, 1], F32)
            nc.vector.memset(c3sq[:], SQ3[3])
            warm = pool.tile([PB, 1], F32)
            # warm-load the tanh table set while the input DMA runs
            nc.scalar.activation(out=warm[:], in_=halfpi[:], func=ACT.Tanh,
                                 scale=2.0)

            cons = pool.tile([PB, 6, A], F32)

            def c(i):
                return _ap(cons, i * A, [[1, A]])

            C_K2Y, C_SA, C_CA, C_SE, C_CE, C_SCR = 0, 1, 2, 3, 4, 5

            consh = pool.tile([PB, 9, A], F16)
            H_P0X, H_P0Y, H_D1, H_D2, H_D3 = 0, 1, 2, 3, 4
            H_CEDT, H_SEDT, H_CADT, H_SADT = 5, 6, 7, 8

            def chb(i):  # broadcast over outer t: [[0,T],[1,A]]
                return _ap(consh, i * A, [[0, T], [1, A]])

            def ch(i):
                return _ap(consh, i * A, [[1, A]])

            scr = pool.tile([PB, 6, A], F32)

            def s(i):
                return _ap(scr, i * A, [[1, A]])

            ki = None
            if not theta_wrap_ok:
                ki = pool.tile([PB, 2, A], I32)

            # ---------------- per-agent constants: DVE-only thunks -----
            # Emitted one per era-1 rollout step (DVE slack); overflow is
            # emitted before the rollout.  No ACT involvement at all.
            thunks = []

            def sincos_dve(theta_fld, out_sin32, out_cos32, sidx):
                tr = s(sidx)
                if theta_wrap_ok:
                    thunks.append(lambda tf=theta_fld, tr=tr:
                                  nc.vector._custom_dve(
                                      ops["WRAP"], out=tr, in0=tf, s0=0.0,
                                      s1=PI, imm2=TWO_PI))
                else:
                    kv = _ap(ki, sidx // 2 * A, [[1, A]])

                    def red(tf=theta_fld, tr=tr, kv=kv):
                        nc.vector.tensor_scalar(out=tr, in0=tf,
                                                scalar1=1.0 / TWO_PI,
                                                scalar2=0.0,
                                                op0=OP.mult, op1=OP.add)
                        nc.vector.tensor_copy(out=kv, in_=tr)
                        nc.vector.tensor_copy(out=tr, in_=kv)
                        nc.vector.scalar_tensor_tensor(
                            out=tr, in0=tr, scalar=-TWO_PI, in1=tf,
                            op0=OP.mult, op1=OP.add)
                    thunks.append(red)
                thunks.append(lambda tr=tr, o=out_sin32:
                              nc.vector._custom_dve(
                                  ops["SIN7_ANT"], out=o, in0=tr,
                                  in1=c3sin[:], s0=SIN7[0], s1=SIN7[1],
                                  imm2=SIN7[2]))
                w = s(sidx + 1)
                thunks.append(lambda tr=tr, w=w:
                              nc.vector._custom_dve(
                                  ops["WRAP"], out=w, in0=tr,
                                  s0=float(np.pi / 2), s1=PI, imm2=TWO_PI))
                thunks.append(lambda w=w, o=out_cos32:
                              nc.vector._custom_dve(
                                  ops["SIN7_ANT"], out=o, in0=w,
                                  in1=c3sin[:], s0=SIN7[0], s1=SIN7[1],
                                  imm2=SIN7[2]))

            sincos_dve(fld(7), c(C_SA), c(C_CA), 0)   # agent theta
            sincos_dve(fld(3), c(C_SE), c(C_CE), 2)   # ego theta
            for src, dst in ((C_CA, H_CADT), (C_SA, H_SADT),
                             (C_CE, H_CEDT), (C_SE, H_SEDT)):
                thunks.append(lambda src=src, dst=dst:
                              nc.vector.tensor_mul(out=ch(dst), in0=c(src),
                                                   in1=fld(14)))
            # re/ra = 0.5*hypot(L, W) via sumsq + cubic
            thunks.append(lambda: nc.vector._custom_dve(
                ops["SUMSQ_ANT"], out=s(4), in0=fld(8), in1=fld(9)))
            thunks.append(lambda: nc.vector._custom_dve(
                ops["POLY3_ANT"], out=s(4), in0=s(4), in1=c3sq[:],
                s0=SQ3[0], s1=SQ3[1], imm2=SQ3[2]))           # re
            thunks.append(lambda: nc.vector._custom_dve(
                ops["SUMSQ_ANT"], out=s(5), in0=fld(11), in1=fld(12)))
            thunks.append(lambda: nc.vector._custom_dve(
                ops["POLY3_ANT"], out=s(5), in0=s(5), in1=c3sq[:],
                s0=SQ3[0], s1=SQ3[1], imm2=SQ3[2]))           # ra
            # d1 = 0.5*(We-Le) f16; d2 = 0.5*(Wa-La) f16
            thunks.append(lambda: nc.vector._custom_dve(
                ops["SUBSCALE_ANT"], out=ch(H_D1), in0=fld(9), in1=fld(8),
                s0=0.5))
            thunks.append(lambda: nc.vector._custom_dve(
                ops["SUBSCALE_ANT"], out=ch(H_D2), in0=fld(12), in1=fld(11),
                s0=0.5))
            # k1y = 0.5*We + ra; k2y = 0.5*Wa + re; d3 = k2y - k1y
            thunks.append(lambda: nc.vector.scalar_tensor_tensor(
                out=s(0), in0=fld(9), scalar=0.5, in1=s(5),
                op0=OP.mult, op1=OP.add))
            thunks.append(lambda: nc.vector.scalar_tensor_tensor(
                out=c(C_K2Y), in0=fld(12), scalar=0.5, in1=s(4),
                op0=OP.mult, op1=OP.add))
            thunks.append(lambda: nc.vector.tensor_sub(
                out=ch(H_D3), in0=c(C_K2Y), in1=s(0)))
            thunks.append(lambda: nc.vector.tensor_sub(
                out=ch(H_P0X), in0=fld(4), in1=fld(0)))
            thunks.append(lambda: nc.vector.tensor_sub(
                out=ch(H_P0Y), in0=fld(5), in1=fld(1)))

            # ---------------- rollout ----------------
            VT = pool.tile([PB, (T + 1) * NC2], F32)
            ST = pool.tile([PB, T * NC2], F16)    # t-major fp16 cumsum

            def vslot(j):  # j=0: strided input view; j in 1..50: contiguous
                if j == 0:
                    return _ap(D, 2, [[4, 2], [F, A]])
                return _ap(VT, j * NC2, [[1, NC2]])

            def stslot(k):  # k in 0..49, t-major contiguous
                return _ap(ST, k * NC2, [[1, NC2]])

            G = pool.tile([PB, NC2], F32)
            nc.vector.tensor_copy(out=stslot(0), in_=vslot(0))

            if dt_uniform is None:
                NDT2 = pool.tile([PB, NC2], F32)
                nc.vector.tensor_scalar_mul(
                    out=NDT2[:], in0=_ap(D, 14, [[0, 2], [F, A]]), scalar1=-9.0)

            MS = KI2 = None
            if k_red > 0 and not v_wrap2_ok:
                MS = pool.tile([PB, NC2 * k_red], F32)
                KI2 = pool.tile([PB, NC2 * k_red], I32)

            VTH = pool.tile([PB, (T + 1) * NC2], F16)  # era-2 fp16 v slots

            def vslot2(j):
                return _ap(VTH, j * NC2, [[1, NC2]])

            SINV = pool.tile([PB, T * NC2], F16)  # slot k = sin(v_{k+1})
            COSV = pool.tile([PB, T * NC2], F16)
            CABS = pool.tile([PB, T * NC2], F16)

            # overflow const thunks run before the rollout
            n_slack = max(0, k_era - (1 if k_red > 0 else 0))
            while len(thunks) > n_slack:
                thunks.pop(0)()

            def step_common(j):
                if j < T:
                    nc.vector.tensor_add(out=stslot(j), in0=stslot(j - 1),
                                         in1=vslot(j))
                if j == k_red and k_red > 0:
                    # range-reduce angle slots 1..k_red in place
                    red_view = _ap(VT, NC2, [[1, NC2 * k_red]])
                    if v_wrap2_ok:  # two chained wraps: valid for |v| < 5*pi
                        for _ in range(2):
                            nc.vector._custom_dve(
                                ops["WRAP"], out=red_view, in0=red_view,
                                s0=0.0, s1=PI, imm2=TWO_PI)
                    else:
                        nc.vector.tensor_scalar_mul(out=MS[:], in0=red_view,
                                                    scalar1=1.0 / TWO_PI)
                        nc.vector.tensor_copy(out=KI2[:], in_=MS[:])
                        nc.vector.tensor_copy(out=MS[:], in_=KI2[:])
                        nc.vector.scalar_tensor_tensor(
                            out=red_view, in0=MS[:], scalar=-TWO_PI,
                            in1=red_view, op0=OP.mult, op1=OP.add)
                elif thunks:
                    thunks.pop(0)()

            # era 1: serial ACT-Tanh + DVE-STT chain
            for j in range(1, k_era + 1):
                nc.scalar.activation(out=G[:], in_=vslot(j - 1),
                                     func=ACT.Tanh, scale=2.0)
                if dt_uniform is None:
                    nc.vector.tensor_mul(out=G[:], in0=G[:], in1=NDT2[:])
                    nc.vector.tensor_add(out=vslot(j), in0=vslot(j - 1),
                                         in1=G[:])
                else:
                    nc.vector.scalar_tensor_tensor(
                        out=vslot(j), in0=G[:], scalar=-9.0 * float(dt_uniform),
                        in1=vslot(j - 1), op0=OP.mult, op1=OP.add)
                step_common(j)

            while thunks:
                thunks.pop(0)()

            # trig prefetch of slots 1..k_era on ACT (idle during era 2)
            npre = k_era if k_era < T else T
            if npre > 0:
                ang_pre = _ap(VT, NC2, [[1, npre * NC2]])
                sin_pre = _ap(SINV, 0, [[1, npre * NC2]])
                cabs_pre = _ap(CABS, 0, [[1, npre * NC2]])
                cos_pre = _ap(COSV, 0, [[1, npre * NC2]])
                nc.scalar.activation(out=sin_pre, in_=ang_pre, func=ACT.Sin)
                nc.scalar.activation(out=cabs_pre, in_=ang_pre, func=ACT.Abs)
                nc.scalar.activation(out=cos_pre, in_=cabs_pre, func=ACT.Sin,
                                     bias=halfpi[:], scale=-1.0)

            # era 2: |v| <= 0.15 -- fused quintic on DVE, no ACT round trip
            # v' = v - 0.9*tanh(2v) ~ v*(-0.8 + v^2*(2.4 - 3.84 v^2))
            for j in range(k_era + 1, T + 1):
                src_v = vslot(j - 1) if j == k_era + 1 else vslot2(j - 1)
                nc.vector._custom_dve(ops["QUINTIC_ANT_V1"], out=vslot2(j),
                                      in0=src_v,
                                      s0=-0.8, s1=2.4, imm2=-3.84)
                if j < T:
                    nc.vector.tensor_add(out=stslot(j), in0=stslot(j - 1),
                                         in1=vslot2(j))
                if thunks:
                    thunks.pop(0)()

            # trig tail: slots k_era+1..T
            if npre < T:
                ntail = T - npre
                ang_tl = _ap(VTH, (npre + 1) * NC2, [[1, ntail * NC2]])
                sin_tl = _ap(SINV, npre * NC2, [[1, ntail * NC2]])
                cabs_tl = _ap(CABS, npre * NC2, [[1, ntail * NC2]])
                cos_tl = _ap(COSV, npre * NC2, [[1, ntail * NC2]])
                nc.scalar.activation(out=sin_tl, in_=ang_tl, func=ACT.Sin)
                nc.scalar.activation(out=cabs_tl, in_=ang_tl, func=ACT.Abs)
                nc.scalar.activation(out=cos_tl, in_=cabs_tl, func=ACT.Sin,
                                     bias=halfpi[:], scale=-1.0)

            # ---------------- distance phase (fp16, t-major) ----------
            SEv = _ap(ST, 0, [[NC2, T], [1, A]])     # ego cumsum
            SAv = _ap(ST, A, [[NC2, T], [1, A]])     # agent cumsum
            SE_ = _ap(SINV, 0, [[NC2, T], [1, A]])   # sin(ve)
            SA_ = _ap(SINV, A, [[NC2, T], [1, A]])
            CE = _ap(COSV, 0, [[NC2, T], [1, A]])
            CA = _ap(COSV, A, [[NC2, T], [1, A]])

            PXY = pool.tile([PB, 2 * NT], F16)
            SCR2 = pool.tile([PB, 2 * NT], F16)
            PX = _ap(PXY, 0, [[1, NT]])
            PY = _ap(PXY, NT, [[1, NT]])
            S1 = _ap(SCR2, 0, [[1, NT]])
            S2 = _ap(SCR2, NT, [[1, NT]])

            nc.vector.tensor_mul(out=S1, in0=SAv, in1=chb(H_CADT))
            nc.vector.tensor_add(out=S1, in0=S1, in1=chb(H_P0X))
            nc.vector.tensor_mul(out=S2, in0=SEv, in1=chb(H_CEDT))
            nc.vector.tensor_sub(out=PX, in0=S1, in1=S2)
            nc.vector.tensor_mul(out=S1, in0=SAv, in1=chb(H_SADT))
            nc.vector.tensor_add(out=S1, in0=S1, in1=chb(H_P0Y))
            nc.vector.tensor_mul(out=S2, in0=SEv, in1=chb(H_SEDT))
            nc.vector.tensor_sub(out=PY, in0=S1, in1=S2)

            # body-frame components; SINV products first (COSV lands later)
            R12 = pool.tile([PB, 2 * NT], F16)
            R34 = pool.tile([PB, 2 * NT], F16)
            R1X = _ap(R12, 0, [[1, NT]])
            R1Y = _ap(R12, NT, [[1, NT]])
            R2X = _ap(R34, 0, [[1, NT]])
            R2Y = _ap(R34, NT, [[1, NT]])

            nc.vector.tensor_mul(out=R1X, in0=SE_, in1=PY)
            nc.vector.tensor_mul(out=R1Y, in0=SE_, in1=PX)
            nc.vector.tensor_mul(out=R2X, in0=SA_, in1=PY)
            nc.vector.tensor_mul(out=R2Y, in0=SA_, in1=PX)
            nc.vector.tensor_mul(out=S1, in0=CE, in1=PX)
            nc.vector.tensor_add(out=R1X, in0=R1X, in1=S1)   # rel1x
            nc.vector.tensor_mul(out=S2, in0=CE, in1=PY)
            nc.vector.tensor_sub(out=R1Y, in0=S2, in1=R1Y)   # rel1y
            nc.vector.tensor_mul(out=S1, in0=CA, in1=PX)
            nc.vector.tensor_add(out=R2X, in0=R2X, in1=S1)   # -rel2x; |.| ok
            nc.vector.tensor_mul(out=S2, in0=CA, in1=PY)
            nc.vector.tensor_sub(out=R2Y, in0=R2Y, in1=S2)   # rel2y

            # |rel| on ACT, then the shifted max-tree:
            # dist = max(max(|r1x|+d1, |r1y|) + d3, max(|r2x|+d2, |r2y|)) - k2y
            for R in (R1X, R1Y, R2X, R2Y):
                nc.scalar.activation(out=R, in_=R, func=ACT.Abs)
            nc.vector.tensor_add(out=R1X, in0=R1X, in1=chb(H_D1))
            nc.vector.tensor_tensor(out=R1X, in0=R1X, in1=R1Y, op=OP.max)
            nc.vector.tensor_add(out=R2X, in0=R2X, in1=chb(H_D2))
            nc.vector.tensor_tensor(out=R2X, in0=R2X, in1=R2Y, op=OP.max)
            nc.vector.tensor_add(out=R1X, in0=R1X, in1=chb(H_D3))
            nc.vector.tensor_tensor(out=R1X, in0=R1X, in1=R2X, op=OP.max)

            # min over t: fp16 pairwise tree on the t-major D = R1X view
            DD = R12

            def dview(k0, n):  # n consecutive t-slots from k0
                return _ap(DD, k0 * A, [[1, n * A]])

            nc.vector.tensor_tensor(out=dview(0, 25), in0=dview(0, 25),
                                    in1=dview(25, 25), op=OP.min)
            nc.vector.tensor_tensor(out=dview(0, 12), in0=dview(0, 12),
                                    in1=dview(12, 12), op=OP.min)
            nc.vector.tensor_tensor(out=dview(0, 6), in0=dview(0, 6),
                                    in1=dview(6, 6), op=OP.min)
            nc.vector.tensor_tensor(out=dview(0, 3), in0=dview(0, 3),
                                    in1=dview(3, 3), op=OP.min)
            nc.vector.tensor_tensor(out=dview(0, 1), in0=dview(0, 1),
                                    in1=dview(1, 1), op=OP.min)
            nc.vector.tensor_tensor(out=dview(0, 1), in0=dview(0, 1),
                                    in1=dview(2, 1), op=OP.min)
            nc.vector.tensor_tensor(out=dview(0, 1), in0=dview(0, 1),
                                    in1=dview(24, 1), op=OP.min)

            H = pool.tile([PB, A], F32)
            nc.vector.tensor_sub(out=H[:], in0=dview(0, 1), in1=c(C_K2Y))
            OUTT = pool.tile([PB, A], F32)
            nc.scalar.activation(out=H[:], in_=H[:], func=ACT.Tanh, scale=0.1)
            nc.vector.tensor_scalar_mul(out=OUTT[:], in0=H[:], scalar1=5.0)
            nc.sync.dma_start(out=out[:], in_=OUTT[:])

    nc.compile()
    return nc


def _get_nc(*params):
    key = ("nc",) + params
    if key not in _cache:
        _cache[key] = _build(*params)
    return _cache[key]


def _make_runner(nc):
    """One-time build of a cached jitted SPMD executable for nc."""
    import jax
    from jax.sharding import Mesh, PartitionSpec
    from jax.experimental.shard_map import shard_map
    from concourse import bass2jax, mybir as _mybir

    bass2jax.install_neuronx_cc_hook()
    partition_name = (nc.partition_id_tensor.name
                      if nc.partition_id_tensor else None)
    in_names, out_names, out_avals, zero_outs = [], [], [], []
    for alloc in nc.m.functions[0].allocations:
        if not isinstance(alloc, _mybir.MemoryLocationSet):
            continue
        name = alloc.memorylocations[0].name
        if alloc.kind == "ExternalInput":
            if name != partition_name:
                in_names.append(name)
        elif alloc.kind == "ExternalOutput":
            shape = tuple(alloc.tensor_shape)
            dtype = _mybir.dt.np(alloc.dtype)
            out_names.append(name)
            out_avals.append(jax.core.ShapedArray(shape, dtype))
            zero_outs.append(np.zeros(shape, dtype))
    n_params = len(in_names)
    all_names = in_names + out_names
    if partition_name is not None:
        all_names = all_names + [partition_name]
    donate = tuple(range(n_params, n_params + len(out_names)))

    def _body(*args):
        operands = list(args)
        if partition_name is not None:
            operands.append(bass2jax.partition_id_tensor())
        outs = bass2jax._bass_exec_p.bind(
            *operands, out_avals=tuple(out_avals), in_names=tuple(all_names),
            out_names=tuple(out_names), lowering_input_output_aliases=(),
            sim_require_finite=True, sim_require_nnan=True, nc=nc)
        return tuple(outs)

    mesh = Mesh(np.asarray(jax.devices()[:N_CORES]), ("core",))
    in_specs = (PartitionSpec("core"),) * (n_params + len(out_names))
    out_specs = (PartitionSpec("core"),) * len(out_names)
    sharded = jax.jit(
        shard_map(_body, mesh=mesh, in_specs=in_specs, out_specs=out_specs,
                  check_rep=False),
        donate_argnums=donate, keep_unused=True)
    concat_zeros = [np.zeros((N_CORES * z.shape[0], *z.shape[1:]), z.dtype)
                    for z in zero_outs]

    def run(full_data_2d):  # [B, A*F] -> [B, A]
        outs = sharded(full_data_2d, *[z.copy() for z in concat_zeros])
        return np.asarray(outs[out_names.index("out")])

    return run


def _params_for(data: np.ndarray):
    dt = data[..., 14]
    dt0 = float(dt.flat[0])
    dt_uniform = dt0 if bool(np.all(dt == dt0)) else None
    vmax = float(np.abs(data[..., [2, 6]]).max())
    # slots j >= k_red have |v_j| <= pi: while |v| > 2.2 each step shrinks
    # |v| by >= 9*dt_min*tanh(4.4), and the map keeps |v| <= pi once below
    # (valid when the max step 9*dt_max <= pi; otherwise reduce every slot).
    dt_min = float(dt.min())
    dt_max = float(dt.max())
    shrink = 9.0 * dt_min * 0.9997
    if 9.0 * dt_max > np.pi or shrink <= 1e-6:
        k_red = T
    else:
        k_red = int(min(T, max(0, np.ceil((vmax - np.pi) / shrink) + 1)))
    # era-2 boundary: for dt ~ 0.1, while |v| >= 1.5 each step shrinks |v|
    # by >= 0.9*tanh(3) = 0.89555; once |v| <= 1.5 three steps of the map
    # v -> v - 0.9*tanh(2v) give |v| <= 0.1406 and |v| <= 0.15 is
    # invariant.  There the odd quintic matches tanh(2v) to 1.4e-5.
    if dt_uniform is not None and abs(dt_uniform - 0.1) < 1e-6:
        k_brake = int(np.ceil(max(0.0, vmax - 1.5) / 0.89555))
        k_era = min(T, max(k_red, k_brake + 3))
    else:
        k_era = T
    # theta range-reduction: single ADD_RANGE_WRAP valid while |theta|<3pi
    thmax = float(np.abs(data[..., [3, 7]]).max())
    theta_wrap_ok = bool(thmax < 3.0 * np.pi - 0.05)
    # v range-reduction via two chained wraps: valid while |v| < 5*pi
    v_wrap2_ok = bool(vmax < 5.0 * np.pi - 0.05)
    # extent hypot cubic validity: u = L^2+W^2 must stay in the fit range
    ee = data[..., 8:10]
    ea = data[..., 11:13]
    u_all = np.concatenate([(ee ** 2).sum(-1).ravel(), (ea ** 2).sum(-1).ravel()])
    if not (SQ3_LO <= float(u_all.min()) and float(u_all.max()) <= SQ3_HI):
        raise ValueError("extent outside sqrt-poly fit range")
    return dt_uniform, k_red, k_era, theta_wrap_ok, v_wrap2_ok


def _run(data: np.ndarray, trace: bool = False):
    data = np.ascontiguousarray(data, dtype=np.float32)
    assert data.shape == (B, A, F), data.shape
    params = _params_for(data)
    nc = _get_nc(*params)
    in_maps = [{"data": data[c * PB:(c + 1) * PB].reshape(PB, A * F)}
               for c in range(N_CORES)]
    res = run_bass_kernel_spmd(nc, in_maps, core_ids=list(range(N_CORES)),
                               trace=trace)
    full = np.concatenate([res.results[c]["out"] for c in range(N_CORES)],
                          axis=0)
    return full, res


def kernel(data: np.ndarray) -> np.ndarray:
    data = np.ascontiguousarray(data, dtype=np.float32)
    assert data.shape == (B, A, F), data.shape
    params = _params_for(data)
    key = ("runner",) + params
    if key not in _cache:
        _cache[key] = _make_runner(_get_nc(*params))
    return _cache[key](data.reshape(B, A * F)).astype(np.float32)


# revision 6
# speedup vs baseline: 1.6864x; 1.0540x over previous
"""Trainium2 Bass kernel for nn_BackupBarrierCBF.

Reference semantics (B=1024, A=64, T=50 unicycle rollout + rect-vs-disc
distance + min-over-horizon + saturation). Crucial subtleties:
  - braking controller: u = (-9*tanh(2*v), 0) => theta is CONSTANT, so
    positions are x0 + cos(theta)*dt*cumsum(v).
  - veh_veh_distance receives traj[..., 0:3] = (x, y, v): the body-frame
    rotation angle is the (time-varying) VELOCITY, not theta.
  - traj slot k holds the state AFTER k+1 steps: position cumsum uses
    v_0..v_k while the stored rotation angle is v_{k+1}.

Per-core structure (batch rows on the 128 partitions), t-major layout
(slot t holds 128 contiguous cols [ego 64 | ag 64]):
  - two-era rollout: era 1 (j<=k_era) is the serial ACT-Tanh + DVE-STT
    chain; era 2 replaces tanh with one fused custom-DVE quintic per
    step (after braking all |v| <= 0.15 where tanh(2v) ~ 2v-8v^3/3+64v^5/15
    to 1.4e-5), freeing ACT to prefetch sin/cos of the early slots.
  - ALL per-agent constants are computed on DVE with custom fused polys
    (deg-7 sine after an ADD_RANGE_WRAP, cos via a second wrap by pi/2;
    sum-of-squares + cubic for 0.5*hypot), producers write fp16 copies
    directly.  ACT's only table sets are tanh (warm-loaded during the
    input DMA) and sin (loaded once during era 2) -- no table load ever
    sits on the critical path.  Most const ops ride in era-1 DVE slack.
  - the cumsum ST is fp16 (one mixed-dtype add per step, hidden under
    the chain latency).
  - distance phase entirely fp16: every tensor_tensor op has packed
    2-byte operands (broadcast constants use outer-stride-0 APs
    [[0,T],[1,A]]), engaging the DVE 2x_1p mode (~0.55 ns/elem).
  - abs on ACT (dtype-independent rate, hidden under DVE); min over the
    horizon via an fp16 pairwise tensor_tensor min tree.

Sharding: pure data parallel over batch B across 8 cores (128 rows/core).
"""
import numpy as np
import concourse.bass as bass
import concourse.bacc as bacc
import concourse.tile as tile
from concourse import mybir
from concourse.bass_utils import run_bass_kernel_spmd

F32 = mybir.dt.float32
F16 = mybir.dt.float16
I32 = mybir.dt.int32
OP = mybir.AluOpType
ACT = mybir.ActivationFunctionType

B, A, F = 1024, 64, 15
N_CORES = 8
PB = B // N_CORES          # 128 batch rows per core (partition dim)
T = 50
NC2 = 2 * A                # 128 columns: [ego agents | other agents]
NT = T * A                 # 3200
TWO_PI = float(2.0 * np.pi)
PI = float(np.pi)

# host-side field-major layout: per row, 15 blocks of 64 agent values,
# ordered so the rollout's inputs (v_e, v_a) form the first 128 columns.
FIELD_ORDER = [2, 6, 0, 1, 3, 4, 5, 7, 8, 9, 10, 11, 12, 13, 14]
FOFF = {f: i * A for i, f in enumerate(FIELD_ORDER)}

# deg-7 odd minimax-ish sine on [-pi, pi]: sin x ~ x*(c0+u*(c1+u*(c2+u*c3)))
SIN7 = (9.98988214e-01, -1.65417177e-01, 7.90467633e-03, -1.41850903e-04)
# cubic for 0.5*sqrt(u) on u in [17, 56] (extent hypot; L~4-6, W~2-4)
SQ3 = (8.59890582e-01, 8.35872232e-02, -8.39524323e-04, 4.74697384e-06)
SQ3_LO, SQ3_HI = 17.0, 56.0

_cache: dict = {}


def _register_ops():
    """Register fused custom DVE ops via the documented OPS.append
    extension point (per-NEFF opcode table rows)."""
    from concourse.dve_spec import (Spec, Src0, Src1, C0, C1, C2, C3, sq,
                                    lower, _spill_c3_to_src1)
    from concourse.dve_ops import (DveOp, OPS, CUSTOM_DVE_SPECS, has_src1,
                                   _SUB_OPCODE_FOR_NAME, _CUSTOM_DVE_ROW_BASE)
    from concourse.dve_uop import DveOpSpec

    made = {}

    def reg(name, body, ref, spill=False):
        if name in _SUB_OPCODE_FOR_NAME:
            made[name] = next(op for op in OPS if op.name == name)
            return
        if spill:
            body = _spill_c3_to_src1(body)
        spec = Spec(body=body, reference=ref)
        shas = {}
        for ver in ("v3", "v4"):
            ds = DveOpSpec(name=name, opcode=0, uops=lower(spec, ver=ver),
                           rd1_en=has_src1(spec))
            shas[ver] = ds.sha(ver)
        op = DveOp(name, spec, subdim=False, uops_sha=shas)
        row = _CUSTOM_DVE_ROW_BASE + len(OPS)
        assert row < 0x20, row
        OPS.append(op)
        _SUB_OPCODE_FOR_NAME[name] = row
        CUSTOM_DVE_SPECS[name] = spec
        made[name] = op

    u = sq(Src0)
    # v' = v*(C0 + v^2*(C1 + v^2*C2)): era-2 tanh step
    reg("QUINTIC_ANT_V1", Src0 * (C0 + u * (C1 + u * C2)),
        lambda in0, in1, s0, s1, imm2:
        in0 * (s0 + in0 * in0 * (s1 + in0 * in0 * imm2)))
    # sin7: x*(C0 + u*(C1 + u*(C2 + u*C3)))  [C3 spilled to Src1]
    reg("SIN7_ANT", Src0 * (C0 + u * (C1 + u * (C2 + u * C3))),
        lambda in0, in1, s0, s1, imm2:
        in0 * (s0 + in0**2 * (s1 + in0**2 * (imm2 + in0**2 * in1))),
        spill=True)
    # poly3: C0 + x*(C1 + x*(C2 + x*C3))  [C3 spilled]
    reg("POLY3_ANT", C0 + Src0 * (C1 + Src0 * (C2 + Src0 * C3)),
        lambda in0, in1, s0, s1, imm2:
        s0 + in0 * (s1 + in0 * (imm2 + in0 * in1)),
        spill=True)
    # sumsq: Src0^2 + Src1^2
    reg("SUMSQ_ANT", sq(Src0) + sq(Src1),
        lambda in0, in1, s0, s1, imm2: in0 * in0 + in1 * in1)
    # subscale: (Src0 - Src1)*C0
    reg("SUBSCALE_ANT", (Src0 - Src1) * C0,
        lambda in0, in1, s0, s1, imm2: (in0 - in1) * s0)
    from concourse.dve_ops import ADD_RANGE_WRAP
    made["WRAP"] = ADD_RANGE_WRAP
    return made


def _ap(t: bass.AP, extra_offset: int, free_dims: list) -> bass.AP:
    """View into tile t: keep partition dim, replace free dims."""
    return bass.AP(tensor=t.tensor, offset=t.offset + extra_offset,
                   ap=[list(t.ap[0])] + [list(d) for d in free_dims])


def _build(dt_uniform, k_red, k_era, theta_wrap_ok, v_wrap2_ok):
    ops = _register_ops()
    nc = bacc.Bacc("TRN2", target_bir_lowering=False)
    data = nc.dram_tensor("data", [PB, A * F], F32, kind="ExternalInput")
    out = nc.dram_tensor("out", [PB, A], F32, kind="ExternalOutput")

    with tile.TileContext(nc) as tc:
        with tc.tile_pool(name="pool", bufs=1) as pool:
            # ---------------- load ----------------
            # field-major: v-block (128 cols) first so the rollout chain
            # starts as soon as the small first DMA lands.
            D = pool.tile([PB, A * F], F32)
            NV = 2 * A
            nc.sync.dma_start(out=_ap(D, 0, [[1, NV]]),
                              in_=_ap(data[:], 0, [[1, NV]]))
            nc.sync.dma_start(out=_ap(D, NV, [[1, A * F - NV]]),
                              in_=_ap(data[:], NV, [[1, A * F - NV]]))

            def fld(k):  # [128, 64] contiguous view of per-agent field k
                return _ap(D, FOFF[k], [[1, A]])

            halfpi = pool.tile([PB, 1], F32)
            nc.vector.memset(halfpi[:], float(np.pi / 2))
            c3sin = pool.tile([PB, 1], F32)
            nc.vector.memset(c3sin[:], SIN7[3])
            c3sq = pool.tile([PB, 1], F32)
            nc.vector.memset(c3sq[:], SQ3[3])
            warm = pool.tile([PB, 1], F32)
            # warm-load the tanh table set while the input DMA runs
            nc.scalar.activation(out=warm[:], in_=halfpi[:], func=ACT.Tanh,
                                 scale=2.0)

            cons = pool.tile([PB, 6, A], F32)

            def c(i):
                return _ap(cons, i * A, [[1, A]])

            C_K2Y, C_SA, C_CA, C_SE, C_CE, C_SCR = 0, 1, 2, 3, 4, 5

            consh = pool.tile([PB, 9, A], F16)
            H_P0X, H_P0Y, H_D1, H_D2, H_D3 = 0, 1, 2, 3, 4
            H_CEDT, H_SEDT, H_CADT, H_SADT = 5, 6, 7, 8

            def chb(i):  # broadcast over outer t: [[0,T],[1,A]]
                return _ap(consh, i * A, [[0, T], [1, A]])

            def ch(i):
                return _ap(consh, i * A, [[1, A]])

            scr = pool.tile([PB, 6, A], F32)

            def s(i):
                return _ap(scr, i * A, [[1, A]])

            ki = None
            if not theta_wrap_ok:
                ki = pool.tile([PB, 2, A], I32)

            # ---------------- per-agent constants: DVE-only thunks -----
            # Emitted one per era-1 rollout step (DVE slack); overflow is
            # emitted before the rollout.  No ACT involvement at all.
            thunks = []

            def sincos_dve(theta_fld, out_sin32, out_cos32, sidx):
                tr = s(sidx)
                if theta_wrap_ok:
                    thunks.append(lambda tf=theta_fld, tr=tr:
                                  nc.vector._custom_dve(
                                      ops["WRAP"], out=tr, in0=tf, s0=0.0,
                                      s1=PI, imm2=TWO_PI))
                else:
                    kv = _ap(ki, sidx // 2 * A, [[1, A]])

                    def red(tf=theta_fld, tr=tr, kv=kv):
                        nc.vector.tensor_scalar(out=tr, in0=tf,
                                                scalar1=1.0 / TWO_PI,
                                                scalar2=0.0,
                                                op0=OP.mult, op1=OP.add)
                        nc.vector.tensor_copy(out=kv, in_=tr)
                        nc.vector.tensor_copy(out=tr, in_=kv)
                        nc.vector.scalar_tensor_tensor(
                            out=tr, in0=tr, scalar=-TWO_PI, in1=tf,
                            op0=OP.mult, op1=OP.add)
                    thunks.append(red)
                thunks.append(lambda tr=tr, o=out_sin32:
                              nc.vector._custom_dve(
                                  ops["SIN7_ANT"], out=o, in0=tr,
                                  in1=c3sin[:], s0=SIN7[0], s1=SIN7[1],
                                  imm2=SIN7[2]))
                w = s(sidx + 1)
                thunks.append(lambda tr=tr, w=w:
                              nc.vector._custom_dve(
                                  ops["WRAP"], out=w, in0=tr,
                                  s0=float(np.pi / 2), s1=PI, imm2=TWO_PI))
                thunks.append(lambda w=w, o=out_cos32:
                              nc.vector._custom_dve(
                                  ops["SIN7_ANT"], out=o, in0=w,
                                  in1=c3sin[:], s0=SIN7[0], s1=SIN7[1],
                                  imm2=SIN7[2]))

            sincos_dve(fld(7), c(C_SA), c(C_CA), 0)   # agent theta
            sincos_dve(fld(3), c(C_SE), c(C_CE), 2)   # ego theta
            for src, dst in ((C_CA, H_CADT), (C_SA, H_SADT),
                             (C_CE, H_CEDT), (C_SE, H_SEDT)):
                thunks.append(lambda src=src, dst=dst:
                              nc.vector.tensor_mul(out=ch(dst), in0=c(src),
                                                   in1=fld(14)))
            # re/ra = 0.5*hypot(L, W) via sumsq + cubic
            thunks.append(lambda: nc.vector._custom_dve(
                ops["SUMSQ_ANT"], out=s(4), in0=fld(8), in1=fld(9)))
            thunks.append(lambda: nc.vector._custom_dve(
                ops["POLY3_ANT"], out=s(4), in0=s(4), in1=c3sq[:],
                s0=SQ3[0], s1=SQ3[1], imm2=SQ3[2]))           # re
            thunks.append(lambda: nc.vector._custom_dve(
                ops["SUMSQ_ANT"], out=s(5), in0=fld(11), in1=fld(12)))
            thunks.append(lambda: nc.vector._custom_dve(
                ops["POLY3_ANT"], out=s(5), in0=s(5), in1=c3sq[:],
                s0=SQ3[0], s1=SQ3[1], imm2=SQ3[2]))           # ra
            # d1 = 0.5*(We-Le) f16; d2 = 0.5*(Wa-La) f16
            thunks.append(lambda: nc.vector._custom_dve(
                ops["SUBSCALE_ANT"], out=ch(H_D1), in0=fld(9), in1=fld(8),
                s0=0.5))
            thunks.append(lambda: nc.vector._custom_dve(
                ops["SUBSCALE_ANT"], out=ch(H_D2), in0=fld(12), in1=fld(11),
                s0=0.5))
            # k1y = 0.5*We + ra; k2y = 0.5*Wa + re; d3 = k2y - k1y
            thunks.append(lambda: nc.vector.scalar_tensor_tensor(
                out=s(0), in0=fld(9), scalar=0.5, in1=s(5),
                op0=OP.mult, op1=OP.add))
            thunks.append(lambda: nc.vector.scalar_tensor_tensor(
                out=c(C_K2Y), in0=fld(12), scalar=0.5, in1=s(4),
                op0=OP.mult, op1=OP.add))
            thunks.append(lambda: nc.vector.tensor_sub(
                out=ch(H_D3), in0=c(C_K2Y), in1=s(0)))
            thunks.append(lambda: nc.vector.tensor_sub(
                out=ch(H_P0X), in0=fld(4), in1=fld(0)))
            thunks.append(lambda: nc.vector.tensor_sub(
                out=ch(H_P0Y), in0=fld(5), in1=fld(1)))

            # ---------------- rollout ----------------
            VT = pool.tile([PB, (T + 1) * NC2], F32)
            ST = pool.tile([PB, T * NC2], F16)    # t-major fp16 cumsum

            def vslot(j):  # j=0: strided input view; j in 1..50: contiguous
                if j == 0:
                    return _ap(D, 0, [[1, NC2]])
                return _ap(VT, j * NC2, [[1, NC2]])

            def stslot(k):  # k in 0..49, t-major contiguous
                return _ap(ST, k * NC2, [[1, NC2]])

            G = pool.tile([PB, NC2], F32)
            nc.vector.tensor_copy(out=stslot(0), in_=vslot(0))

            if dt_uniform is None:
                NDT2 = pool.tile([PB, NC2], F32)
                nc.vector.tensor_scalar_mul(
                    out=NDT2[:], in0=_ap(D, FOFF[14], [[0, 2], [1, A]]),
                    scalar1=-9.0)

            MS = KI2 = None
            if k_red > 0 and not v_wrap2_ok:
                MS = pool.tile([PB, NC2 * k_red], F32)
                KI2 = pool.tile([PB, NC2 * k_red], I32)

            VTH = pool.tile([PB, (T + 1) * NC2], F16)  # era-2 fp16 v slots

            def vslot2(j):
                return _ap(VTH, j * NC2, [[1, NC2]])

            SINV = pool.tile([PB, T * NC2], F16)  # slot k = sin(v_{k+1})
            COSV = pool.tile([PB, T * NC2], F16)
            CABS = pool.tile([PB, T * NC2], F16)

            THUNK_START = 5   # steps 1..4 run before DMA2 lands

            def step_common(j):
                if j < T:
                    nc.vector.tensor_add(out=stslot(j), in0=stslot(j - 1),
                                         in1=vslot(j))
                if j == k_red and k_red > 0:
                    # range-reduce angle slots 1..k_red in place
                    red_view = _ap(VT, NC2, [[1, NC2 * k_red]])
                    if v_wrap2_ok:  # two chained wraps: valid for |v| < 5*pi
                        # slots >= k1 already have |v| <= 3*pi: one wrap
                        k1 = min(k_red, max(1, k_red - 9))
                        hot = _ap(VT, NC2, [[1, NC2 * k1]])
                        nc.vector._custom_dve(
                            ops["WRAP"], out=hot, in0=hot,
                            s0=0.0, s1=PI, imm2=TWO_PI)
                        nc.vector._custom_dve(
                            ops["WRAP"], out=red_view, in0=red_view,
                            s0=0.0, s1=PI, imm2=TWO_PI)
                    else:
                        nc.vector.tensor_scalar_mul(out=MS[:], in0=red_view,
                                                    scalar1=1.0 / TWO_PI)
                        nc.vector.tensor_copy(out=KI2[:], in_=MS[:])
                        nc.vector.tensor_copy(out=MS[:], in_=KI2[:])
                        nc.vector.scalar_tensor_tensor(
                            out=red_view, in0=MS[:], scalar=-TWO_PI,
                            in1=red_view, op0=OP.mult, op1=OP.add)
                elif thunks and j >= THUNK_START:
                    thunks.pop(0)()
                    if thunks and j >= THUNK_START + 8:
                        thunks.pop(0)()

            # era 1: serial ACT-Tanh + DVE-STT chain
            for j in range(1, k_era + 1):
                nc.scalar.activation(out=G[:], in_=vslot(j - 1),
                                     func=ACT.Tanh, scale=2.0)
                if dt_uniform is None:
                    nc.vector.tensor_mul(out=G[:], in0=G[:], in1=NDT2[:])
                    nc.vector.tensor_add(out=vslot(j), in0=vslot(j - 1),
                                         in1=G[:])
                else:
                    nc.vector.scalar_tensor_tensor(
                        out=vslot(j), in0=G[:], scalar=-9.0 * float(dt_uniform),
                        in1=vslot(j - 1), op0=OP.mult, op1=OP.add)
                step_common(j)

            while thunks:
                thunks.pop(0)()

            # trig prefetch of slots 1..k_era on ACT (idle during era 2)
            npre = k_era if k_era < T else T
            if npre > 0:
                ang_pre = _ap(VT, NC2, [[1, npre * NC2]])
                sin_pre = _ap(SINV, 0, [[1, npre * NC2]])
                cabs_pre = _ap(CABS, 0, [[1, npre * NC2]])
                cos_pre = _ap(COSV, 0, [[1, npre * NC2]])
                nc.scalar.activation(out=sin_pre, in_=ang_pre, func=ACT.Sin)
                nc.scalar.activation(out=cabs_pre, in_=ang_pre, func=ACT.Abs)
                nc.scalar.activation(out=cos_pre, in_=cabs_pre, func=ACT.Sin,
                                     bias=halfpi[:], scale=-1.0)

            # era 2: |v| <= 0.15 -- fused quintic on DVE, no ACT round trip
            # v' = v - 0.9*tanh(2v) ~ v*(-0.8 + v^2*(2.4 - 3.84 v^2))
            for j in range(k_era + 1, T + 1):
                src_v = vslot(j - 1) if j == k_era + 1 else vslot2(j - 1)
                nc.vector._custom_dve(ops["QUINTIC_ANT_V1"], out=vslot2(j),
                                      in0=src_v,
                                      s0=-0.8, s1=2.4, imm2=-3.84)
                if j < T:
                    nc.vector.tensor_add(out=stslot(j), in0=stslot(j - 1),
                                         in1=vslot2(j))
                if thunks:
                    thunks.pop(0)()

            # trig tail: slots k_era+1..T
            if npre < T:
                ntail = T - npre
                ang_tl = _ap(VTH, (npre + 1) * NC2, [[1, ntail * NC2]])
                sin_tl = _ap(SINV, npre * NC2, [[1, ntail * NC2]])
                cabs_tl = _ap(CABS, npre * NC2, [[1, ntail * NC2]])
                cos_tl = _ap(COSV, npre * NC2, [[1, ntail * NC2]])
                nc.scalar.activation(out=sin_tl, in_=ang_tl, func=ACT.Sin)
                nc.scalar.activation(out=cabs_tl, in_=ang_tl, func=ACT.Abs)
                nc.scalar.activation(out=cos_tl, in_=cabs_tl, func=ACT.Sin,
                                     bias=halfpi[:], scale=-1.0)

            # ---------------- distance phase (fp16, t-major) ----------
            SEv = _ap(ST, 0, [[NC2, T], [1, A]])     # ego cumsum
            SAv = _ap(ST, A, [[NC2, T], [1, A]])     # agent cumsum
            SE_ = _ap(SINV, 0, [[NC2, T], [1, A]])   # sin(ve)
            SA_ = _ap(SINV, A, [[NC2, T], [1, A]])
            CE = _ap(COSV, 0, [[NC2, T], [1, A]])
            CA = _ap(COSV, A, [[NC2, T], [1, A]])

            PXY = pool.tile([PB, 2 * NT], F16)
            SCR2 = pool.tile([PB, 2 * NT], F16)
            PX = _ap(PXY, 0, [[1, NT]])
            PY = _ap(PXY, NT, [[1, NT]])
            S1 = _ap(SCR2, 0, [[1, NT]])
            S2 = _ap(SCR2, NT, [[1, NT]])

            nc.vector.tensor_mul(out=S1, in0=SAv, in1=chb(H_CADT))
            nc.vector.tensor_add(out=S1, in0=S1, in1=chb(H_P0X))
            nc.vector.tensor_mul(out=S2, in0=SEv, in1=chb(H_CEDT))
            nc.vector.tensor_sub(out=PX, in0=S1, in1=S2)
            nc.vector.tensor_mul(out=S1, in0=SAv, in1=chb(H_SADT))
            nc.vector.tensor_add(out=S1, in0=S1, in1=chb(H_P0Y))
            nc.vector.tensor_mul(out=S2, in0=SEv, in1=chb(H_SEDT))
            nc.vector.tensor_sub(out=PY, in0=S1, in1=S2)

            # body-frame components; SINV products first (COSV lands later)
            R12 = pool.tile([PB, 2 * NT], F16)
            R34 = pool.tile([PB, 2 * NT], F16)
            R1X = _ap(R12, 0, [[1, NT]])
            R1Y = _ap(R12, NT, [[1, NT]])
            R2X = _ap(R34, 0, [[1, NT]])
            R2Y = _ap(R34, NT, [[1, NT]])

            nc.vector.tensor_mul(out=R1X, in0=SE_, in1=PY)
            nc.vector.tensor_mul(out=R1Y, in0=SE_, in1=PX)
            nc.vector.tensor_mul(out=R2X, in0=SA_, in1=PY)
            nc.vector.tensor_mul(out=R2Y, in0=SA_, in1=PX)
            nc.vector.tensor_mul(out=S1, in0=CE, in1=PX)
            nc.vector.tensor_add(out=R1X, in0=R1X, in1=S1)   # rel1x
            nc.vector.tensor_mul(out=S2, in0=CE, in1=PY)
            nc.vector.tensor_sub(out=R1Y, in0=S2, in1=R1Y)   # rel1y
            nc.vector.tensor_mul(out=S1, in0=CA, in1=PX)
            nc.vector.tensor_add(out=R2X, in0=R2X, in1=S1)   # -rel2x; |.| ok
            nc.vector.tensor_mul(out=S2, in0=CA, in1=PY)
            nc.vector.tensor_sub(out=R2Y, in0=R2Y, in1=S2)   # rel2y

            # |rel| on ACT, then the shifted max-tree:
            # dist = max(max(|r1x|+d1, |r1y|) + d3, max(|r2x|+d2, |r2y|)) - k2y
            for R in (R1X, R1Y, R2X, R2Y):
                nc.scalar.activation(out=R, in_=R, func=ACT.Abs)
            nc.vector.tensor_add(out=R1X, in0=R1X, in1=chb(H_D1))
            nc.vector.tensor_tensor(out=R1X, in0=R1X, in1=R1Y, op=OP.max)
            nc.vector.tensor_add(out=R2X, in0=R2X, in1=chb(H_D2))
            nc.vector.tensor_tensor(out=R2X, in0=R2X, in1=R2Y, op=OP.max)
            nc.vector.tensor_add(out=R1X, in0=R1X, in1=chb(H_D3))
            nc.vector.tensor_tensor(out=R1X, in0=R1X, in1=R2X, op=OP.max)

            # min over t: fp16 pairwise tree on the t-major D = R1X view
            DD = R12

            def dview(k0, n):  # n consecutive t-slots from k0
                return _ap(DD, k0 * A, [[1, n * A]])

            nc.vector.tensor_tensor(out=dview(0, 25), in0=dview(0, 25),
                                    in1=dview(25, 25), op=OP.min)
            nc.vector.tensor_tensor(out=dview(0, 12), in0=dview(0, 12),
                                    in1=dview(12, 12), op=OP.min)
            nc.vector.tensor_tensor(out=dview(0, 6), in0=dview(0, 6),
                                    in1=dview(6, 6), op=OP.min)
            nc.vector.tensor_tensor(out=dview(0, 3), in0=dview(0, 3),
                                    in1=dview(3, 3), op=OP.min)
            nc.vector.tensor_tensor(out=dview(0, 1), in0=dview(0, 1),
                                    in1=dview(1, 1), op=OP.min)
            nc.vector.tensor_tensor(out=dview(0, 1), in0=dview(0, 1),
                                    in1=dview(2, 1), op=OP.min)
            nc.vector.tensor_tensor(out=dview(0, 1), in0=dview(0, 1),
                                    in1=dview(24, 1), op=OP.min)

            H = pool.tile([PB, A], F32)
            nc.vector.tensor_sub(out=H[:], in0=dview(0, 1), in1=c(C_K2Y))
            OUTT = pool.tile([PB, A], F32)
            nc.scalar.activation(out=H[:], in_=H[:], func=ACT.Tanh, scale=0.1)
            nc.vector.tensor_scalar_mul(out=OUTT[:], in0=H[:], scalar1=5.0)
            nc.sync.dma_start(out=out[:], in_=OUTT[:])

    nc.compile()
    return nc


def _get_nc(*params):
    key = ("nc",) + params
    if key not in _cache:
        _cache[key] = _build(*params)
    return _cache[key]


def _make_runner(nc):
    """One-time build of a cached jitted SPMD executable for nc."""
    import jax
    from jax.sharding import Mesh, PartitionSpec
    from jax.experimental.shard_map import shard_map
    from concourse import bass2jax, mybir as _mybir

    bass2jax.install_neuronx_cc_hook()
    partition_name = (nc.partition_id_tensor.name
                      if nc.partition_id_tensor else None)
    in_names, out_names, out_avals, zero_outs = [], [], [], []
    for alloc in nc.m.functions[0].allocations:
        if not isinstance(alloc, _mybir.MemoryLocationSet):
            continue
        name = alloc.memorylocations[0].name
        if alloc.kind == "ExternalInput":
            if name != partition_name:
                in_names.append(name)
        elif alloc.kind == "ExternalOutput":
            shape = tuple(alloc.tensor_shape)
            dtype = _mybir.dt.np(alloc.dtype)
            out_names.append(name)
            out_avals.append(jax.core.ShapedArray(shape, dtype))
            zero_outs.append(np.zeros(shape, dtype))
    n_params = len(in_names)
    all_names = in_names + out_names
    if partition_name is not None:
        all_names = all_names + [partition_name]
    donate = tuple(range(n_params, n_params + len(out_names)))

    def _body(*args):
        operands = list(args)
        if partition_name is not None:
            operands.append(bass2jax.partition_id_tensor())
        outs = bass2jax._bass_exec_p.bind(
            *operands, out_avals=tuple(out_avals), in_names=tuple(all_names),
            out_names=tuple(out_names), lowering_input_output_aliases=(),
            sim_require_finite=True, sim_require_nnan=True, nc=nc)
        return tuple(outs)

    mesh = Mesh(np.asarray(jax.devices()[:N_CORES]), ("core",))
    in_specs = (PartitionSpec("core"),) * (n_params + len(out_names))
    out_specs = (PartitionSpec("core"),) * len(out_names)
    sharded = jax.jit(
        shard_map(_body, mesh=mesh, in_specs=in_specs, out_specs=out_specs,
                  check_rep=False),
        donate_argnums=donate, keep_unused=True)
    concat_zeros = [np.zeros((N_CORES * z.shape[0], *z.shape[1:]), z.dtype)
                    for z in zero_outs]

    def run(full_data_2d):  # [B, A*F] -> [B, A]
        outs = sharded(full_data_2d, *[z.copy() for z in concat_zeros])
        return np.asarray(outs[out_names.index("out")])

    return run


def _params_for(data: np.ndarray):
    dt = data[..., 14]
    dt0 = float(dt.flat[0])
    dt_uniform = dt0 if bool(np.all(dt == dt0)) else None
    vmax = float(np.abs(data[..., [2, 6]]).max())
    # slots j >= k_red have |v_j| <= pi: while |v| > 2.2 each step shrinks
    # |v| by >= 9*dt_min*tanh(4.4), and the map keeps |v| <= pi once below
    # (valid when the max step 9*dt_max <= pi; otherwise reduce every slot).
    dt_min = float(dt.min())
    dt_max = float(dt.max())
    shrink = 9.0 * dt_min * 0.9997
    if 9.0 * dt_max > np.pi or shrink <= 1e-6:
        k_red = T
    else:
        k_red = int(min(T, max(0, np.ceil((vmax - np.pi) / shrink) + 1)))
    # era-2 boundary: for dt ~ 0.1, while |v| >= 1.5 each step shrinks |v|
    # by >= 0.9*tanh(3) = 0.89555; once |v| <= 1.5 three steps of the map
    # v -> v - 0.9*tanh(2v) give |v| <= 0.1406 and |v| <= 0.15 is
    # invariant.  There the odd quintic matches tanh(2v) to 1.4e-5.
    if dt_uniform is not None and abs(dt_uniform - 0.1) < 1e-6:
        k_brake = int(np.ceil(max(0.0, vmax - 1.5) / 0.89555))
        k_era = min(T, max(k_red, k_brake + 3))
    else:
        k_era = T
    # theta range-reduction: single ADD_RANGE_WRAP valid while |theta|<3pi
    thmax = float(np.abs(data[..., [3, 7]]).max())
    theta_wrap_ok = bool(thmax < 3.0 * np.pi - 0.05)
    # v range-reduction via two chained wraps: valid while |v| < 5*pi
    v_wrap2_ok = bool(vmax < 5.0 * np.pi - 0.05)
    # extent hypot cubic validity: u = L^2+W^2 must stay in the fit range
    ee = data[..., 8:10]
    ea = data[..., 11:13]
    u_all = np.concatenate([(ee ** 2).sum(-1).ravel(), (ea ** 2).sum(-1).ravel()])
    if not (SQ3_LO <= float(u_all.min()) and float(u_all.max()) <= SQ3_HI):
        raise ValueError("extent outside sqrt-poly fit range")
    return dt_uniform, k_red, k_era, theta_wrap_ok, v_wrap2_ok


def _run(data: np.ndarray, trace: bool = False):
    data = np.ascontiguousarray(data, dtype=np.float32)
    assert data.shape == (B, A, F), data.shape
    params = _params_for(data)
    nc = _get_nc(*params)
    dfm = np.ascontiguousarray(
        data.transpose(0, 2, 1)[:, FIELD_ORDER, :]).reshape(B, A * F)
    in_maps = [{"data": dfm[c * PB:(c + 1) * PB]} for c in range(N_CORES)]
    res = run_bass_kernel_spmd(nc, in_maps, core_ids=list(range(N_CORES)),
                               trace=trace)
    full = np.concatenate([res.results[c]["out"] for c in range(N_CORES)],
                          axis=0)
    return full, res


def kernel(data: np.ndarray) -> np.ndarray:
    data = np.ascontiguousarray(data, dtype=np.float32)
    assert data.shape == (B, A, F), data.shape
    params = _params_for(data)
    key = ("runner",) + params
    if key not in _cache:
        _cache[key] = _make_runner(_get_nc(*params))
    dfm = np.ascontiguousarray(
        data.transpose(0, 2, 1)[:, FIELD_ORDER, :]).reshape(B, A * F)
    return _cache[key](dfm).astype(np.float32)


# revision 7
# speedup vs baseline: 1.6965x; 1.0059x over previous
"""Trainium2 Bass kernel for nn_BackupBarrierCBF.

Reference semantics (B=1024, A=64, T=50 unicycle rollout + rect-vs-disc
distance + min-over-horizon + saturation). Crucial subtleties:
  - braking controller: u = (-9*tanh(2*v), 0) => theta is CONSTANT, so
    positions are x0 + cos(theta)*dt*cumsum(v).
  - veh_veh_distance receives traj[..., 0:3] = (x, y, v): the body-frame
    rotation angle is the (time-varying) VELOCITY, not theta.
  - traj slot k holds the state AFTER k+1 steps: position cumsum uses
    v_0..v_k while the stored rotation angle is v_{k+1}.

Per-core structure (batch rows on the 128 partitions), t-major layout
(slot t holds 128 contiguous cols [ego 64 | ag 64]):
  - two-era rollout: era 1 (j<=k_era) is the serial ACT-Tanh + DVE-STT
    chain; era 2 replaces tanh with one fused custom-DVE quintic per
    step (after braking all |v| <= 0.15 where tanh(2v) ~ 2v-8v^3/3+64v^5/15
    to 1.4e-5), freeing ACT to prefetch sin/cos of the early slots.
  - ALL per-agent constants are computed on DVE with custom fused polys
    (deg-7 sine after an ADD_RANGE_WRAP, cos via a second wrap by pi/2;
    sum-of-squares + cubic for 0.5*hypot), producers write fp16 copies
    directly.  ACT's only table sets are tanh (warm-loaded during the
    input DMA) and sin (loaded once during era 2) -- no table load ever
    sits on the critical path.  Most const ops ride in era-1 DVE slack.
  - the cumsum ST is fp16 (one mixed-dtype add per step, hidden under
    the chain latency).
  - distance phase entirely fp16: every tensor_tensor op has packed
    2-byte operands (broadcast constants use outer-stride-0 APs
    [[0,T],[1,A]]), engaging the DVE 2x_1p mode (~0.55 ns/elem).
  - abs on ACT (dtype-independent rate, hidden under DVE); min over the
    horizon via an fp16 pairwise tensor_tensor min tree.

Sharding: pure data parallel over batch B across 8 cores (128 rows/core).
"""
import numpy as np
import concourse.bass as bass
import concourse.bacc as bacc
import concourse.tile as tile
from concourse import mybir
from concourse.bass_utils import run_bass_kernel_spmd

F32 = mybir.dt.float32
F16 = mybir.dt.float16
I32 = mybir.dt.int32
OP = mybir.AluOpType
ACT = mybir.ActivationFunctionType

B, A, F = 1024, 64, 15
N_CORES = 8
PB = B // N_CORES          # 128 batch rows per core (partition dim)
T = 50
NC2 = 2 * A                # 128 columns: [ego agents | other agents]
NT = T * A                 # 3200
TWO_PI = float(2.0 * np.pi)
PI = float(np.pi)

# host-side field-major layout: per row, 15 blocks of 64 agent values,
# ordered so the rollout's inputs (v_e, v_a) form the first 128 columns.
FIELD_ORDER = [2, 6, 0, 1, 3, 4, 5, 7, 8, 9, 10, 11, 12, 13, 14]
FOFF = {f: i * A for i, f in enumerate(FIELD_ORDER)}

# deg-7 odd minimax-ish sine on [-pi, pi]: sin x ~ x*(c0+u*(c1+u*(c2+u*c3)))
SIN7 = (9.98988214e-01, -1.65417177e-01, 7.90467633e-03, -1.41850903e-04)
# deg-7 odd fit of the era-2 map g(v) = v - 0.9*tanh(2v) on [-0.66, 0.66]
G7 = (-0.7958460572929034, 2.2731611766802913, -2.695966128739347,
      1.6363191974497997)
# cubic for 0.5*sqrt(u) on u in [17, 56] (extent hypot; L~4-6, W~2-4)
SQ3 = (8.59890582e-01, 8.35872232e-02, -8.39524323e-04, 4.74697384e-06)
SQ3_LO, SQ3_HI = 17.0, 56.0

_cache: dict = {}


def _register_ops():
    """Register fused custom DVE ops via the documented OPS.append
    extension point (per-NEFF opcode table rows)."""
    from concourse.dve_spec import (Spec, Src0, Src1, C0, C1, C2, C3, sq,
                                    lower, _spill_c3_to_src1)
    from concourse.dve_ops import (DveOp, OPS, CUSTOM_DVE_SPECS, has_src1,
                                   _SUB_OPCODE_FOR_NAME, _CUSTOM_DVE_ROW_BASE)
    from concourse.dve_uop import DveOpSpec

    made = {}

    def reg(name, body, ref, spill=False):
        if name in _SUB_OPCODE_FOR_NAME:
            made[name] = next(op for op in OPS if op.name == name)
            return
        if spill:
            body = _spill_c3_to_src1(body)
        spec = Spec(body=body, reference=ref)
        shas = {}
        for ver in ("v3", "v4"):
            ds = DveOpSpec(name=name, opcode=0, uops=lower(spec, ver=ver),
                           rd1_en=has_src1(spec))
            shas[ver] = ds.sha(ver)
        op = DveOp(name, spec, subdim=False, uops_sha=shas)
        row = _CUSTOM_DVE_ROW_BASE + len(OPS)
        assert row < 0x20, row
        OPS.append(op)
        _SUB_OPCODE_FOR_NAME[name] = row
        CUSTOM_DVE_SPECS[name] = spec
        made[name] = op

    u = sq(Src0)
    # v' = v*(C0 + v^2*(C1 + v^2*C2)): era-2 tanh step
    reg("QUINTIC_ANT_V1", Src0 * (C0 + u * (C1 + u * C2)),
        lambda in0, in1, s0, s1, imm2:
        in0 * (s0 + in0 * in0 * (s1 + in0 * in0 * imm2)))
    # sin7: x*(C0 + u*(C1 + u*(C2 + u*C3)))  [C3 spilled to Src1]
    reg("SIN7_ANT", Src0 * (C0 + u * (C1 + u * (C2 + u * C3))),
        lambda in0, in1, s0, s1, imm2:
        in0 * (s0 + in0**2 * (s1 + in0**2 * (imm2 + in0**2 * in1))),
        spill=True)
    # poly3: C0 + x*(C1 + x*(C2 + x*C3))  [C3 spilled]
    reg("POLY3_ANT", C0 + Src0 * (C1 + Src0 * (C2 + Src0 * C3)),
        lambda in0, in1, s0, s1, imm2:
        s0 + in0 * (s1 + in0 * (imm2 + in0 * in1)),
        spill=True)
    # sumsq: Src0^2 + Src1^2
    reg("SUMSQ_ANT", sq(Src0) + sq(Src1),
        lambda in0, in1, s0, s1, imm2: in0 * in0 + in1 * in1)
    # subscale: (Src0 - Src1)*C0
    reg("SUBSCALE_ANT", (Src0 - Src1) * C0,
        lambda in0, in1, s0, s1, imm2: (in0 - in1) * s0)
    from concourse.dve_ops import ADD_RANGE_WRAP
    made["WRAP"] = ADD_RANGE_WRAP
    return made


def _ap(t: bass.AP, extra_offset: int, free_dims: list) -> bass.AP:
    """View into tile t: keep partition dim, replace free dims."""
    return bass.AP(tensor=t.tensor, offset=t.offset + extra_offset,
                   ap=[list(t.ap[0])] + [list(d) for d in free_dims])


def _build(dt_uniform, k_red, k_era, theta_wrap_ok, v_wrap2_ok):
    ops = _register_ops()
    nc = bacc.Bacc("TRN2", target_bir_lowering=False)
    data = nc.dram_tensor("data", [PB, A * F], F32, kind="ExternalInput")
    out = nc.dram_tensor("out", [PB, A], F32, kind="ExternalOutput")

    with tile.TileContext(nc) as tc:
        with tc.tile_pool(name="pool", bufs=1) as pool:
            # ---------------- load ----------------
            # field-major: v-block (128 cols) first so the rollout chain
            # starts as soon as the small first DMA lands.
            D = pool.tile([PB, A * F], F32)
            NV = 2 * A
            nc.sync.dma_start(out=_ap(D, 0, [[1, NV]]),
                              in_=_ap(data[:], 0, [[1, NV]]))
            nc.sync.dma_start(out=_ap(D, NV, [[1, A * F - NV]]),
                              in_=_ap(data[:], NV, [[1, A * F - NV]]))

            def fld(k):  # [128, 64] contiguous view of per-agent field k
                return _ap(D, FOFF[k], [[1, A]])

            halfpi = pool.tile([PB, 1], F32)
            nc.vector.memset(halfpi[:], float(np.pi / 2))
            c3sin = pool.tile([PB, 1], F32)
            dtu = float(dt_uniform) if dt_uniform is not None else 1.0
            nc.vector.memset(c3sin[:], SIN7[3] * dtu)
            c3g = pool.tile([PB, 1], F32)
            nc.vector.memset(c3g[:], G7[3])
            c3sq = pool.tile([PB, 1], F32)
            nc.vector.memset(c3sq[:], SQ3[3])
            warm = pool.tile([PB, 1], F32)
            # warm-load the tanh table set while the input DMA runs
            nc.scalar.activation(out=warm[:], in_=halfpi[:], func=ACT.Tanh,
                                 scale=2.0)

            cons = pool.tile([PB, 6, A], F32)

            def c(i):
                return _ap(cons, i * A, [[1, A]])

            C_K2Y, C_SA, C_CA, C_SE, C_CE, C_SCR = 0, 1, 2, 3, 4, 5

            consh = pool.tile([PB, 9, A], F16)
            H_P0X, H_P0Y, H_D1, H_D2, H_D3 = 0, 1, 2, 3, 4
            H_CEDT, H_SEDT, H_CADT, H_SADT = 5, 6, 7, 8

            def chb(i):  # broadcast over outer t: [[0,T],[1,A]]
                return _ap(consh, i * A, [[0, T], [1, A]])

            def ch(i):
                return _ap(consh, i * A, [[1, A]])

            scr = pool.tile([PB, 6, A], F32)

            def s(i):
                return _ap(scr, i * A, [[1, A]])

            ki = None
            if not theta_wrap_ok:
                ki = pool.tile([PB, 2, A], I32)

            # ---------------- per-agent constants: DVE-only thunks -----
            # Emitted one per era-1 rollout step (DVE slack); overflow is
            # emitted before the rollout.  No ACT involvement at all.
            thunks = []

            def sincos_dve(theta_fld, out_sin, out_cos, sidx):
                tr = s(sidx)
                if theta_wrap_ok:
                    thunks.append(lambda tf=theta_fld, tr=tr:
                                  nc.vector._custom_dve(
                                      ops["WRAP"], out=tr, in0=tf, s0=0.0,
                                      s1=PI, imm2=TWO_PI))
                else:
                    kv = _ap(ki, sidx // 2 * A, [[1, A]])

                    def red(tf=theta_fld, tr=tr, kv=kv):
                        nc.vector.tensor_scalar(out=tr, in0=tf,
                                                scalar1=1.0 / TWO_PI,
                                                scalar2=0.0,
                                                op0=OP.mult, op1=OP.add)
                        nc.vector.tensor_copy(out=kv, in_=tr)
                        nc.vector.tensor_copy(out=tr, in_=kv)
                        nc.vector.scalar_tensor_tensor(
                            out=tr, in0=tr, scalar=-TWO_PI, in1=tf,
                            op0=OP.mult, op1=OP.add)
                    thunks.append(red)
                thunks.append(lambda tr=tr, o=out_sin:
                              nc.vector._custom_dve(
                                  ops["SIN7_ANT"], out=o, in0=tr,
                                  in1=c3sin[:], s0=SIN7[0] * dtu,
                                  s1=SIN7[1] * dtu, imm2=SIN7[2] * dtu))
                w = s(sidx + 1)
                thunks.append(lambda tr=tr, w=w:
                              nc.vector._custom_dve(
                                  ops["WRAP"], out=w, in0=tr,
                                  s0=float(np.pi / 2), s1=PI, imm2=TWO_PI))
                thunks.append(lambda w=w, o=out_cos:
                              nc.vector._custom_dve(
                                  ops["SIN7_ANT"], out=o, in0=w,
                                  in1=c3sin[:], s0=SIN7[0] * dtu,
                                  s1=SIN7[1] * dtu, imm2=SIN7[2] * dtu))

            if dt_uniform is not None:
                # dt folded into the sine-poly coefficients: write the
                # fp16 *dt constants directly
                sincos_dve(fld(7), ch(H_SADT), ch(H_CADT), 0)
                sincos_dve(fld(3), ch(H_SEDT), ch(H_CEDT), 2)
            else:
                sincos_dve(fld(7), c(C_SA), c(C_CA), 0)
                sincos_dve(fld(3), c(C_SE), c(C_CE), 2)
                for csrc, dst in ((C_CA, H_CADT), (C_SA, H_SADT),
                                  (C_CE, H_CEDT), (C_SE, H_SEDT)):
                    thunks.append(lambda csrc=csrc, dst=dst:
                                  nc.vector.tensor_mul(out=ch(dst),
                                                       in0=c(csrc),
                                                       in1=fld(14)))
            # re/ra = 0.5*hypot(L, W) via sumsq + cubic
            thunks.append(lambda: nc.vector._custom_dve(
                ops["SUMSQ_ANT"], out=s(4), in0=fld(8), in1=fld(9)))
            thunks.append(lambda: nc.vector._custom_dve(
                ops["POLY3_ANT"], out=s(4), in0=s(4), in1=c3sq[:],
                s0=SQ3[0], s1=SQ3[1], imm2=SQ3[2]))           # re
            thunks.append(lambda: nc.vector._custom_dve(
                ops["SUMSQ_ANT"], out=s(5), in0=fld(11), in1=fld(12)))
            thunks.append(lambda: nc.vector._custom_dve(
                ops["POLY3_ANT"], out=s(5), in0=s(5), in1=c3sq[:],
                s0=SQ3[0], s1=SQ3[1], imm2=SQ3[2]))           # ra
            # d1 = 0.5*(We-Le) f16; d2 = 0.5*(Wa-La) f16
            thunks.append(lambda: nc.vector._custom_dve(
                ops["SUBSCALE_ANT"], out=ch(H_D1), in0=fld(9), in1=fld(8),
                s0=0.5))
            thunks.append(lambda: nc.vector._custom_dve(
                ops["SUBSCALE_ANT"], out=ch(H_D2), in0=fld(12), in1=fld(11),
                s0=0.5))
            # k1y = 0.5*We + ra; k2y = 0.5*Wa + re; d3 = k2y - k1y
            thunks.append(lambda: nc.vector.scalar_tensor_tensor(
                out=s(0), in0=fld(9), scalar=0.5, in1=s(5),
                op0=OP.mult, op1=OP.add))
            thunks.append(lambda: nc.vector.scalar_tensor_tensor(
                out=c(C_K2Y), in0=fld(12), scalar=0.5, in1=s(4),
                op0=OP.mult, op1=OP.add))
            thunks.append(lambda: nc.vector.tensor_sub(
                out=ch(H_D3), in0=c(C_K2Y), in1=s(0)))
            thunks.append(lambda: nc.vector.tensor_sub(
                out=ch(H_P0X), in0=fld(4), in1=fld(0)))
            thunks.append(lambda: nc.vector.tensor_sub(
                out=ch(H_P0Y), in0=fld(5), in1=fld(1)))

            # ---------------- rollout ----------------
            VT = pool.tile([PB, (T + 1) * NC2], F32)
            ST = pool.tile([PB, T * NC2], F16)    # t-major fp16 cumsum

            def vslot(j):  # j=0: strided input view; j in 1..50: contiguous
                if j == 0:
                    return _ap(D, 0, [[1, NC2]])
                return _ap(VT, j * NC2, [[1, NC2]])

            def stslot(k):  # k in 0..49, t-major contiguous
                return _ap(ST, k * NC2, [[1, NC2]])

            G = pool.tile([PB, NC2], F32)
            nc.vector.tensor_copy(out=stslot(0), in_=vslot(0))

            if dt_uniform is None:
                NDT2 = pool.tile([PB, NC2], F32)
                nc.vector.tensor_scalar_mul(
                    out=NDT2[:], in0=_ap(D, FOFF[14], [[0, 2], [1, A]]),
                    scalar1=-9.0)

            MS = KI2 = None
            if k_red > 0 and not v_wrap2_ok:
                MS = pool.tile([PB, NC2 * k_red], F32)
                KI2 = pool.tile([PB, NC2 * k_red], I32)

            VTH = pool.tile([PB, (T + 1) * NC2], F16)  # era-2 fp16 v slots

            def vslot2(j):
                return _ap(VTH, j * NC2, [[1, NC2]])

            SINV = pool.tile([PB, T * NC2], F16)  # slot k = sin(v_{k+1})
            COSV = pool.tile([PB, T * NC2], F16)
            CABS = pool.tile([PB, T * NC2], F16)

            THUNK_START = 5   # steps 1..4 run before DMA2 lands

            def step_common(j):
                if j < T:
                    nc.vector.tensor_add(out=stslot(j), in0=stslot(j - 1),
                                         in1=vslot(j))
                if j == k_red and k_red > 0:
                    # range-reduce angle slots 1..k_red in place
                    red_view = _ap(VT, NC2, [[1, NC2 * k_red]])
                    if v_wrap2_ok:  # two chained wraps: valid for |v| < 5*pi
                        # slots >= k1 already have |v| <= 3*pi: one wrap
                        k1 = min(k_red, max(1, k_red - 9))
                        hot = _ap(VT, NC2, [[1, NC2 * k1]])
                        nc.vector._custom_dve(
                            ops["WRAP"], out=hot, in0=hot,
                            s0=0.0, s1=PI, imm2=TWO_PI)
                        nc.vector._custom_dve(
                            ops["WRAP"], out=red_view, in0=red_view,
                            s0=0.0, s1=PI, imm2=TWO_PI)
                    else:
                        nc.vector.tensor_scalar_mul(out=MS[:], in0=red_view,
                                                    scalar1=1.0 / TWO_PI)
                        nc.vector.tensor_copy(out=KI2[:], in_=MS[:])
                        nc.vector.tensor_copy(out=MS[:], in_=KI2[:])
                        nc.vector.scalar_tensor_tensor(
                            out=red_view, in0=MS[:], scalar=-TWO_PI,
                            in1=red_view, op0=OP.mult, op1=OP.add)
                elif thunks and j >= THUNK_START:
                    thunks.pop(0)()
                    if thunks and j >= THUNK_START + 8:
                        thunks.pop(0)()

            # era 1: serial ACT-Tanh + DVE-STT chain
            for j in range(1, k_era + 1):
                nc.scalar.activation(out=G[:], in_=vslot(j - 1),
                                     func=ACT.Tanh, scale=2.0)
                if dt_uniform is None:
                    nc.vector.tensor_mul(out=G[:], in0=G[:], in1=NDT2[:])
                    nc.vector.tensor_add(out=vslot(j), in0=vslot(j - 1),
                                         in1=G[:])
                else:
                    nc.vector.scalar_tensor_tensor(
                        out=vslot(j), in0=G[:], scalar=-9.0 * float(dt_uniform),
                        in1=vslot(j - 1), op0=OP.mult, op1=OP.add)
                step_common(j)

            while thunks:
                thunks.pop(0)()

            # trig prefetch of slots 1..k_era on ACT (idle during era 2)
            npre = k_era if k_era < T else T
            if npre > 0:
                ang_pre = _ap(VT, NC2, [[1, npre * NC2]])
                sin_pre = _ap(SINV, 0, [[1, npre * NC2]])
                cabs_pre = _ap(CABS, 0, [[1, npre * NC2]])
                cos_pre = _ap(COSV, 0, [[1, npre * NC2]])
                nc.scalar.activation(out=sin_pre, in_=ang_pre, func=ACT.Sin)
                nc.scalar.activation(out=cabs_pre, in_=ang_pre, func=ACT.Abs)
                nc.scalar.activation(out=cos_pre, in_=cabs_pre, func=ACT.Sin,
                                     bias=halfpi[:], scale=-1.0)

            # era 2: |v| <= 0.15 -- fused quintic on DVE, no ACT round trip
            # v' = v - 0.9*tanh(2v) ~ v*(-0.8 + v^2*(2.4 - 3.84 v^2))
            for j in range(k_era + 1, T + 1):
                src_v = vslot(j - 1) if j == k_era + 1 else vslot2(j - 1)
                nc.vector._custom_dve(ops["SIN7_ANT"], out=vslot2(j),
                                      in0=src_v, in1=c3g[:],
                                      s0=G7[0], s1=G7[1], imm2=G7[2])
                if j < T:
                    nc.vector.tensor_add(out=stslot(j), in0=stslot(j - 1),
                                         in1=vslot2(j))
                if thunks:
                    thunks.pop(0)()

            # trig tail: slots k_era+1..T
            if npre < T:
                ntail = T - npre
                ang_tl = _ap(VTH, (npre + 1) * NC2, [[1, ntail * NC2]])
                sin_tl = _ap(SINV, npre * NC2, [[1, ntail * NC2]])
                cabs_tl = _ap(CABS, npre * NC2, [[1, ntail * NC2]])
                cos_tl = _ap(COSV, npre * NC2, [[1, ntail * NC2]])
                nc.scalar.activation(out=sin_tl, in_=ang_tl, func=ACT.Sin)
                nc.scalar.activation(out=cabs_tl, in_=ang_tl, func=ACT.Abs)
                nc.scalar.activation(out=cos_tl, in_=cabs_tl, func=ACT.Sin,
                                     bias=halfpi[:], scale=-1.0)

            # ---------------- distance phase (fp16, t-major) ----------
            SEv = _ap(ST, 0, [[NC2, T], [1, A]])     # ego cumsum
            SAv = _ap(ST, A, [[NC2, T], [1, A]])     # agent cumsum
            SE_ = _ap(SINV, 0, [[NC2, T], [1, A]])   # sin(ve)
            SA_ = _ap(SINV, A, [[NC2, T], [1, A]])
            CE = _ap(COSV, 0, [[NC2, T], [1, A]])
            CA = _ap(COSV, A, [[NC2, T], [1, A]])

            PXY = pool.tile([PB, 2 * NT], F16)
            SCR2 = pool.tile([PB, 2 * NT], F16)
            PX = _ap(PXY, 0, [[1, NT]])
            PY = _ap(PXY, NT, [[1, NT]])
            S1 = _ap(SCR2, 0, [[1, NT]])
            S2 = _ap(SCR2, NT, [[1, NT]])

            nc.vector.tensor_mul(out=S1, in0=SAv, in1=chb(H_CADT))
            nc.vector.tensor_add(out=S1, in0=S1, in1=chb(H_P0X))
            nc.vector.tensor_mul(out=S2, in0=SEv, in1=chb(H_CEDT))
            nc.vector.tensor_sub(out=PX, in0=S1, in1=S2)
            nc.vector.tensor_mul(out=S1, in0=SAv, in1=chb(H_SADT))
            nc.vector.tensor_add(out=S1, in0=S1, in1=chb(H_P0Y))
            nc.vector.tensor_mul(out=S2, in0=SEv, in1=chb(H_SEDT))
            nc.vector.tensor_sub(out=PY, in0=S1, in1=S2)

            # body-frame components; SINV products first (COSV lands later)
            R12 = pool.tile([PB, 2 * NT], F16)
            R34 = pool.tile([PB, 2 * NT], F16)
            R1X = _ap(R12, 0, [[1, NT]])
            R1Y = _ap(R12, NT, [[1, NT]])
            R2X = _ap(R34, 0, [[1, NT]])
            R2Y = _ap(R34, NT, [[1, NT]])

            nc.vector.tensor_mul(out=R1X, in0=SE_, in1=PY)
            nc.vector.tensor_mul(out=R1Y, in0=SE_, in1=PX)
            nc.vector.tensor_mul(out=R2X, in0=SA_, in1=PY)
            nc.vector.tensor_mul(out=R2Y, in0=SA_, in1=PX)
            nc.vector.tensor_mul(out=S1, in0=CE, in1=PX)
            nc.vector.tensor_add(out=R1X, in0=R1X, in1=S1)   # rel1x
            nc.vector.tensor_mul(out=S2, in0=CE, in1=PY)
            nc.vector.tensor_sub(out=R1Y, in0=S2, in1=R1Y)   # rel1y
            nc.vector.tensor_mul(out=S1, in0=CA, in1=PX)
            nc.vector.tensor_add(out=R2X, in0=R2X, in1=S1)   # -rel2x; |.| ok
            nc.vector.tensor_mul(out=S2, in0=CA, in1=PY)
            nc.vector.tensor_sub(out=R2Y, in0=R2Y, in1=S2)   # rel2y

            # |rel| on ACT, then the shifted max-tree:
            # dist = max(max(|r1x|+d1, |r1y|) + d3, max(|r2x|+d2, |r2y|)) - k2y
            for R in (R1X, R1Y, R2X, R2Y):
                nc.scalar.activation(out=R, in_=R, func=ACT.Abs)
            nc.vector.tensor_add(out=R1X, in0=R1X, in1=chb(H_D1))
            nc.vector.tensor_tensor(out=R1X, in0=R1X, in1=R1Y, op=OP.max)
            nc.vector.tensor_add(out=R2X, in0=R2X, in1=chb(H_D2))
            nc.vector.tensor_tensor(out=R2X, in0=R2X, in1=R2Y, op=OP.max)
            nc.vector.tensor_add(out=R1X, in0=R1X, in1=chb(H_D3))
            nc.vector.tensor_tensor(out=R1X, in0=R1X, in1=R2X, op=OP.max)

            # min over t: fp16 pairwise tree on the t-major D = R1X view
            DD = R12

            def dview(k0, n):  # n consecutive t-slots from k0
                return _ap(DD, k0 * A, [[1, n * A]])

            nc.vector.tensor_tensor(out=dview(0, 25), in0=dview(0, 25),
                                    in1=dview(25, 25), op=OP.min)
            nc.vector.tensor_tensor(out=dview(0, 12), in0=dview(0, 12),
                                    in1=dview(12, 12), op=OP.min)
            nc.vector.tensor_tensor(out=dview(0, 6), in0=dview(0, 6),
                                    in1=dview(6, 6), op=OP.min)
            nc.vector.tensor_tensor(out=dview(0, 3), in0=dview(0, 3),
                                    in1=dview(3, 3), op=OP.min)
            nc.vector.tensor_tensor(out=dview(0, 1), in0=dview(0, 1),
                                    in1=dview(1, 1), op=OP.min)
            nc.vector.tensor_tensor(out=dview(0, 1), in0=dview(0, 1),
                                    in1=dview(2, 1), op=OP.min)
            nc.vector.tensor_tensor(out=dview(0, 1), in0=dview(0, 1),
                                    in1=dview(24, 1), op=OP.min)

            H = pool.tile([PB, A], F32)
            nc.vector.tensor_sub(out=H[:], in0=dview(0, 1), in1=c(C_K2Y))
            OUTT = pool.tile([PB, A], F32)
            nc.scalar.activation(out=H[:], in_=H[:], func=ACT.Tanh, scale=0.1)
            nc.vector.tensor_scalar_mul(out=OUTT[:], in0=H[:], scalar1=5.0)
            nc.sync.dma_start(out=out[:], in_=OUTT[:])

    nc.compile()
    return nc


def _get_nc(*params):
    key = ("nc",) + params
    if key not in _cache:
        _cache[key] = _build(*params)
    return _cache[key]


def _make_runner(nc):
    """One-time build of a cached jitted SPMD executable for nc."""
    import jax
    from jax.sharding import Mesh, PartitionSpec
    from jax.experimental.shard_map import shard_map
    from concourse import bass2jax, mybir as _mybir

    bass2jax.install_neuronx_cc_hook()
    partition_name = (nc.partition_id_tensor.name
                      if nc.partition_id_tensor else None)
    in_names, out_names, out_avals, zero_outs = [], [], [], []
    for alloc in nc.m.functions[0].allocations:
        if not isinstance(alloc, _mybir.MemoryLocationSet):
            continue
        name = alloc.memorylocations[0].name
        if alloc.kind == "ExternalInput":
            if name != partition_name:
                in_names.append(name)
        elif alloc.kind == "ExternalOutput":
            shape = tuple(alloc.tensor_shape)
            dtype = _mybir.dt.np(alloc.dtype)
            out_names.append(name)
            out_avals.append(jax.core.ShapedArray(shape, dtype))
            zero_outs.append(np.zeros(shape, dtype))
    n_params = len(in_names)
    all_names = in_names + out_names
    if partition_name is not None:
        all_names = all_names + [partition_name]
    donate = tuple(range(n_params, n_params + len(out_names)))

    def _body(*args):
        operands = list(args)
        if partition_name is not None:
            operands.append(bass2jax.partition_id_tensor())
        outs = bass2jax._bass_exec_p.bind(
            *operands, out_avals=tuple(out_avals), in_names=tuple(all_names),
            out_names=tuple(out_names), lowering_input_output_aliases=(),
            sim_require_finite=True, sim_require_nnan=True, nc=nc)
        return tuple(outs)

    mesh = Mesh(np.asarray(jax.devices()[:N_CORES]), ("core",))
    in_specs = (PartitionSpec("core"),) * (n_params + len(out_names))
    out_specs = (PartitionSpec("core"),) * len(out_names)
    sharded = jax.jit(
        shard_map(_body, mesh=mesh, in_specs=in_specs, out_specs=out_specs,
                  check_rep=False),
        donate_argnums=donate, keep_unused=True)
    concat_zeros = [np.zeros((N_CORES * z.shape[0], *z.shape[1:]), z.dtype)
                    for z in zero_outs]

    def run(full_data_2d):  # [B, A*F] -> [B, A]
        outs = sharded(full_data_2d, *[z.copy() for z in concat_zeros])
        return np.asarray(outs[out_names.index("out")])

    return run


def _params_for(data: np.ndarray):
    dt = data[..., 14]
    dt0 = float(dt.flat[0])
    dt_uniform = dt0 if bool(np.all(dt == dt0)) else None
    vmax = float(np.abs(data[..., [2, 6]]).max())
    # slots j >= k_red have |v_j| <= pi: while |v| > 2.2 each step shrinks
    # |v| by >= 9*dt_min*tanh(4.4), and the map keeps |v| <= pi once below
    # (valid when the max step 9*dt_max <= pi; otherwise reduce every slot).
    dt_min = float(dt.min())
    dt_max = float(dt.max())
    shrink = 9.0 * dt_min * 0.9997
    if 9.0 * dt_max > np.pi or shrink <= 1e-6:
        k_red = T
    else:
        k_red = int(min(T, max(0, np.ceil((vmax - np.pi) / shrink) + 1)))
    # era-2 boundary: for dt ~ 0.1, while |v| >= 1.5 each step shrinks |v|
    # by >= 0.9*tanh(3) = 0.89555; once |v| <= 1.5 three steps of the map
    # v -> v - 0.9*tanh(2v) give |v| <= 0.1406 and |v| <= 0.15 is
    # invariant.  There the odd quintic matches tanh(2v) to 1.4e-5.
    if dt_uniform is not None and abs(dt_uniform - 0.1) < 1e-6:
        k_brake = int(np.ceil(max(0.0, vmax - 1.5) / 0.89555))
        k_era = min(T, max(k_red, k_brake + 1))
    else:
        k_era = T
    # theta range-reduction: single ADD_RANGE_WRAP valid while |theta|<3pi
    thmax = float(np.abs(data[..., [3, 7]]).max())
    theta_wrap_ok = bool(thmax < 3.0 * np.pi - 0.05)
    # v range-reduction via two chained wraps: valid while |v| < 5*pi
    v_wrap2_ok = bool(vmax < 5.0 * np.pi - 0.05)
    # extent hypot cubic validity: u = L^2+W^2 must stay in the fit range
    ee = data[..., 8:10]
    ea = data[..., 11:13]
    u_all = np.concatenate([(ee ** 2).sum(-1).ravel(), (ea ** 2).sum(-1).ravel()])
    if not (SQ3_LO <= float(u_all.min()) and float(u_all.max()) <= SQ3_HI):
        raise ValueError("extent outside sqrt-poly fit range")
    return dt_uniform, k_red, k_era, theta_wrap_ok, v_wrap2_ok


def _run(data: np.ndarray, trace: bool = False):
    data = np.ascontiguousarray(data, dtype=np.float32)
    assert data.shape == (B, A, F), data.shape
    params = _params_for(data)
    nc = _get_nc(*params)
    dfm = np.ascontiguousarray(
        data.transpose(0, 2, 1)[:, FIELD_ORDER, :]).reshape(B, A * F)
    in_maps = [{"data": dfm[c * PB:(c + 1) * PB]} for c in range(N_CORES)]
    res = run_bass_kernel_spmd(nc, in_maps, core_ids=list(range(N_CORES)),
                               trace=trace)
    full = np.concatenate([res.results[c]["out"] for c in range(N_CORES)],
                          axis=0)
    return full, res


def kernel(data: np.ndarray) -> np.ndarray:
    data = np.ascontiguousarray(data, dtype=np.float32)
    assert data.shape == (B, A, F), data.shape
    params = _params_for(data)
    key = ("runner",) + params
    if key not in _cache:
        _cache[key] = _make_runner(_get_nc(*params))
    dfm = np.ascontiguousarray(
        data.transpose(0, 2, 1)[:, FIELD_ORDER, :]).reshape(B, A * F)
    return _cache[key](dfm).astype(np.float32)
